# revision 1
# baseline (speedup 1.0000x reference)
"""Trainium2 Bass kernel for nn_BoxesFromMasks (per-frame segment bounding boxes).

Algorithm (per core, data-parallel over frames):
  For each frame, build per-pixel one-hot bitmasks of the instance id using an
  exponent-bit trick (int ops construct the bit pattern of float 2^k, an ACT
  copy casts float->uint32 which truncates out-of-range ids to 0):
    lo plane: id s in [0,32)  -> bit (31-s)
    hi plane: id s in [32,64) -> bit (s-32)
  Row masks:  OR-reduce each 128-row chunk along the free (column) axis.
  Col masks:  OR-accumulate chunks into a per-column accumulator, then
              DMA-transpose (as uint16) and OR-reduce along rows.
  Extraction: expand mask bits per id with constant tables, select coordinate
              values, min/max reduce, and partition-fold to one partition.
"""

import numpy as np

_T, _H, _W, _N = 16, 1024, 2048, 64
_NCORES = 8

_BUILD_CACHE = {}


def _build_program(TL, H, W, split_waits=True, reps=1, dbg=False):
    from contextlib import ExitStack

    import concourse.bass as bass
    import concourse.tile as tile
    import concourse.mybir as mybir
    from concourse.alu_op_type import AluOpType as Op

    f32 = mybir.dt.float32
    i32 = mybir.dt.int32
    u32 = mybir.dt.uint32
    u16 = mybir.dt.uint16
    Copy = mybir.ActivationFunctionType.Copy
    X = mybir.AxisListType.X

    P = 128
    CH = H // P                   # row chunks per frame
    UC = 2 * W                    # u16 columns per plane
    KT = 8 if UC % (128 * 8) == 0 else UC // 128   # transpose DMA splits
    SPLIT = UC // KT              # u16 cols per transpose call
    MPER = SPLIT // 128           # mid-dim blocks per call
    B = UC // 128                 # total transposed blocks (unused)
    BH = W // 128                 # blocks per halfword table
    BIG = 0x7FFF                  # absent sentinel (fits i16, fp32-exact)
    BIG16 = 0x7FFF

    # ---- constant tables ----
    pp = np.arange(P)
    yv = (np.arange(CH)[None, :] * P + pp[:, None]).astype(np.int64)    # [P, CH]
    bb = np.arange(B)
    xv = ((SPLIT // 2) * (bb[None, :] // MPER) + 64 * (bb[None, :] % MPER)
          + (pp[:, None] % 64)).astype(np.int64)                        # [P, B]
    # select-value scalars, fp32 (tensor_scalar AP scalars must be fp32;
    # every value is < 2^15 so fp32 arithmetic on them is exact)
    tables = {
        "ymB": (yv - BIG).astype(np.float32),
        "yp1": (yv + 1).astype(np.float32),
        "xmB": (xv - BIG16).astype(np.float32),
        "xp1": (xv + 1).astype(np.float32),
    }

    nc = bass.Bass()
    seg_in = nc.dram_tensor("seg", [TL, H, W], i32, kind="ExternalInput")
    boxes_out = nc.dram_tensor("boxes", [TL, 64, 4], f32, kind="ExternalOutput")

    cmbounce = nc.dram_tensor("cmbounce", [TL, P, 2, B], u16)
    d_ymB, d_yp1, d_xmB, d_xp1 = (
        nc.dram_tensor(n, list(tables[n].shape), f32, kind="ExternalInput")
        for n in ["ymB", "yp1", "xmB", "xp1"])

    if dbg:
        dbg_rmask = nc.dram_tensor("dbg_rmask", [P, TL, 2, CH], u32,
                                   kind="ExternalOutput")
        dbg_cmask = nc.dram_tensor("dbg_cmask", [P, TL, 2, 2, B], u16,
                                   kind="ExternalOutput")
        dbg_E32 = nc.dram_tensor("dbg_E32", [P, 2, 32, TL, CH], i32,
                                 kind="ExternalOutput")
        dbg_E16 = nc.dram_tensor("dbg_E16", [P, 2, 32, TL, B], mybir.dt.int16,
                                 kind="ExternalOutput")
        dbg_SR = nc.dram_tensor("dbg_SR", [P, 4], mybir.dt.int16,
                                kind="ExternalOutput")

    with tile.TileContext(nc) as tc, ExitStack() as ctx:
        constp = ctx.enter_context(tc.tile_pool(name="consts", bufs=1))
        segp = ctx.enter_context(tc.tile_pool(name="segp", bufs=2))
        ep = ctx.enter_context(tc.tile_pool(name="ep", bufs=3))
        accp = ctx.enter_context(tc.tile_pool(name="accp", bufs=2))
        accTp = ctx.enter_context(tc.tile_pool(name="accTp", bufs=1))
        maskp = ctx.enter_context(tc.tile_pool(name="maskp", bufs=1))
        xp = ctx.enter_context(tc.tile_pool(name="xp", bufs=2))
        trp = ctx.enter_context(tc.tile_pool(name="trp", bufs=2))
        smallp = ctx.enter_context(tc.tile_pool(name="smallp", bufs=1))

        c_ymB = constp.tile([P, CH], f32)
        nc.sync.dma_start(c_ymB[:], d_ymB[:])
        c_yp1 = constp.tile([P, CH], f32)
        nc.sync.dma_start(c_yp1[:], d_yp1[:])
        c_xmB = constp.tile([P, B], f32)
        nc.sync.dma_start(c_xmB[:], d_xmB[:])
        c_xp1 = constp.tile([P, B], f32)
        nc.sync.dma_start(c_xp1[:], d_xp1[:])

        # body repeated `reps` times (identical output; used for wall-clock
        # device-time measurement: (wall(R) - wall(1)) / (R - 1))
        for _rep in range(reps):
            rmask16 = maskp.tile([P, TL, 2, CH, 16], u32, tag="rmask16")
            cmask = maskp.tile([P, TL, 2, 2, B], u16, tag="cmask")

            # ================= main loop =================
            for f in range(TL):
                acc = accp.tile([P, 2, W], u32)
                prev_u = None
                for c in range(CH):
                    s = segp.tile([P, W], i32)
                    for k in range(8):
                        nc.sync.dma_start(
                            s[16 * k:16 * (k + 1), :],
                            seg_in[f, c * P + 16 * k:c * P + 16 * (k + 1), :])

                    e = ep.tile([P, 2, W], i32)
                    # lo: bitpattern of 2^(31-s) = (158-s)<<23 ; hi: 2^(s-32) = (s+95)<<23
                    nc.scalar.activation(e[:, 0, :], s[:], Copy,
                                         bias=1325400064.0, scale=-8388608.0)
                    nc.gpsimd.tensor_scalar(e[:, 1, :], s[:], 8388608, 796917760,
                                            Op.mult, Op.add)
                    u = e[:].bitcast(u32)  # in-place cast target
                    nc.scalar.activation(u, e[:].bitcast(f32), Copy)

                    # column accumulate (DVE; only DVE has integer bitwise ops)
                    if c == 0:
                        prev_u = u
                    elif c == 1:
                        nc.vector.tensor_tensor(acc[:], u, prev_u, Op.bitwise_or)
                    else:
                        nc.vector.tensor_tensor(acc[:], u, acc[:], Op.bitwise_or)

                    # row masks: OR-tree along columns (DVE). In place, except
                    # chunk 0 whose u must stay intact for the c==1 accumulate.
                    if c == 0:
                        tr0 = trp.tile([P, 2, W // 2], u32, tag="tr0")
                        base = tr0[:]
                    else:
                        base = e[:, :, 0:W // 2].bitcast(u32)
                    w = W // 2
                    nc.vector.tensor_tensor(base[:, :, 0:w], u[:, :, 0:w],
                                            u[:, :, w:2 * w], Op.bitwise_or)
                    w //= 2
                    while w >= 16:
                        nc.vector.tensor_tensor(base[:, :, 0:w], base[:, :, 0:w],
                                                base[:, :, w:2 * w], Op.bitwise_or)
                        w //= 2
                    # leftovers [P, 2, 16] -> rmask16 wide buffer; folded later
                    nc.vector.tensor_copy(rmask16[:, f, :, c, :], base[:, :, 0:16])

                # ---- transpose acc as u16 and OR-reduce rows; then
                # parity-sort partitions so halfword planes are contiguous:
                # u16col = k*SPLIT + 128*m + p, so halfword h = p & 1.
                accT = accTp.tile([P, 2, B, 128], u16)
                for pl in range(2):
                    a16 = acc[:, pl, :].bitcast(u16)   # [P, UC]
                    for k in range(KT):
                        nc.sync.dma_start(accT[:, pl, k * MPER:(k + 1) * MPER, :],
                                          a16[:, k * SPLIT:(k + 1) * SPLIT],
                                          transpose=True)
                w = 64
                while w >= 1:
                    nc.vector.tensor_tensor(accT[:, :, :, 0:w], accT[:, :, :, 0:w],
                                            accT[:, :, :, w:2 * w], Op.bitwise_or)
                    w //= 2
                # cmask[q, f, pl, h, b]: q<64 <-> p=2q (h=0), q>=64 <-> p=2q+1
                # partition parity sort via a small DRAM bounce
                cmtmp = smallp.tile([P, 2, B], u16, tag="cmtmp")
                nc.vector.tensor_copy(cmtmp[:], accT[:, :, :, 0])
                nc.sync.dma_start(cmbounce[f], cmtmp[:])
                cb = cmbounce[f].rearrange("(q two) a b -> q two a b", two=2)
                for h in range(2):
                    nc.sync.dma_start(cmask[64 * h:64 * (h + 1), f, :, h, :],
                                      cb[:, h, :, :])

            # ================= extraction =================
            i16 = mybir.dt.int16
            assert 2 * 32 * TL == 128  # per-stat slot block == one transpose column set

            # ISA APs allow at most 3 free dims: expand per plane, then flatten
            # (pl, s', f) -> one 128-wide dim for the value-select and reduce.
            def flat1(t):
                return t[:].rearrange("p a b c d -> p (a b c d)")

            def flat3(t):
                return t[:].rearrange("p a b c d -> p (a b c) d")

            # fold rowmask leftovers [.., 16] -> [.., 1]
            rmf = rmask16[:].rearrange("p a b c w -> p (a b c) w")
            w = 8
            while w >= 1:
                nc.vector.tensor_tensor(rmf[:, :, 0:w], rmf[:, :, 0:w],
                                        rmf[:, :, w:2 * w], Op.bitwise_or)
                w //= 2

            if dbg:
                nc.sync.dma_start(dbg_rmask[:], rmask16[:, :, :, :, 0])
                nc.sync.dma_start(dbg_cmask[:], cmask[:])

            # ---- row side: ymin / ymax ----
            # E = (mask >> bit) & 1  (one op per id slot; int immediates)
            E32 = xp.tile([P, 2, 32, TL, CH], i32, tag="xE")
            for pl in range(2):
                rm_v = rmask16[:, :, pl, :, 0]            # [P, TL, CH]
                for sp in range(32):
                    bit = (31 - sp) if pl == 0 else sp
                    nc.vector.tensor_scalar(
                        E32[:, pl, sp], rm_v.bitcast(i32), bit, 1,
                        Op.logical_shift_right, Op.bitwise_and)

            if dbg:
                nc.sync.dma_start(dbg_E32[:], E32[:])

            # cmin = E*(v-BIG) + BIG in {v, BIG}; cmax = E*(v+1) in {v+1, 0}
            cmin32 = xp.tile([P, 2, 32, TL, CH], i32, tag="xc")
            cmax32 = xp.tile([P, 2, 32, TL, CH], i32, tag="xc")
            for c in range(CH):
                nc.scalar.activation(
                    cmin32[:, :, :, :, c], E32[:, :, :, :, c], Copy,
                    scale=c_ymB[:, c].unsqueeze(1), bias=float(BIG))
                nc.scalar.activation(
                    cmax32[:, :, :, :, c], E32[:, :, :, :, c], Copy,
                    scale=c_yp1[:, c].unsqueeze(1), bias=0.0)

            rmin = smallp.tile([P, 2, 32, TL], i32)
            rmax = smallp.tile([P, 2, 32, TL], i32)
            rmin_f = rmin[:].rearrange("p a b f -> p (a b f)")
            rmax_f = rmax[:].rearrange("p a b f -> p (a b f)")
            nc.vector.tensor_reduce(rmin_f, flat3(cmin32), axis=X, op=Op.min)
            nc.vector.tensor_reduce(rmax_f, flat3(cmax32), axis=X, op=Op.max)

            # ---- col side: xmin / xmax ----
            # each slot's bits live in one parity-half of the partitions;
            # zero the rest so they stay neutral through the select.
            E16 = xp.tile([P, 2, 32, TL, B], i16, tag="xE")
            nc.gpsimd.memset(E16[:], 0)
            for pl in range(2):
                for sp in range(32):
                    bit = (31 - sp) if pl == 0 else sp
                    h_req, inbit = bit >> 4, bit & 15
                    q0 = 64 * h_req
                    cm_v = cmask[q0:q0 + 64, :, pl, h_req, :]   # [64, TL, B]
                    nc.vector.tensor_scalar(
                        E16[q0:q0 + 64, pl, sp], cm_v.bitcast(i16), inbit, 1,
                        Op.logical_shift_right, Op.bitwise_and)

            if dbg:
                nc.sync.dma_start(dbg_E16[:], E16[:])

            cmin16 = xp.tile([P, 2, 32, TL, B], i16, tag="xc")
            cmax16 = xp.tile([P, 2, 32, TL, B], i16, tag="xc")
            for b in range(B):
                nc.scalar.activation(
                    cmin16[:, :, :, :, b], E16[:, :, :, :, b], Copy,
                    scale=c_xmB[:, b].unsqueeze(1), bias=float(BIG16))
                nc.scalar.activation(
                    cmax16[:, :, :, :, b], E16[:, :, :, :, b], Copy,
                    scale=c_xp1[:, b].unsqueeze(1), bias=0.0)

            # combined signed stat tile: S[p, k, pl, s', f], k: 0=-xmin 1=-ymin
            # 2=xmax+1 3=ymax+1 (max-fold works for all four)
            S = smallp.tile([P, 4, 2, 32, TL], i16)

            def srow(k, dt=None):
                ap = S[:, k].rearrange("p a b f -> p (a b f)")
                return ap.bitcast(dt) if dt is not None else ap

            nc.vector.tensor_reduce(srow(0), flat3(cmin16), axis=X, op=Op.min)
            nc.vector.tensor_scalar(srow(0), srow(0), -1, 0, Op.mult, Op.add)
            nc.vector.tensor_copy(srow(1), rmin_f)
            nc.vector.tensor_scalar(srow(1), srow(1), -1, 0, Op.mult, Op.add)
            nc.vector.tensor_reduce(srow(2), flat3(cmax16), axis=X, op=Op.max)
            nc.vector.tensor_copy(srow(3), rmax_f)

            # partition fold via u16 DMA transpose + X-reduce over source partitions
            S2 = S[:].rearrange("p k a b f -> p (k a b f)")   # [128, 512]
            ST = smallp.tile([P, 4, 128], i16)
            for m in range(4):
                nc.sync.dma_start(ST[:, m, :], S2[:, 128 * m:128 * (m + 1)],
                                  transpose=True)
            SR = smallp.tile([P, 4], i16)
            nc.vector.tensor_reduce(SR[:], ST[:], axis=X, op=Op.max)

            if dbg:
                nc.sync.dma_start(dbg_SR[:], SR[:])

            # finalize: V[p, k] with p = (pl*32+s')*TL + f
            V = smallp.tile([P, 4], i32)
            nc.vector.tensor_copy(V[:], SR[:])
            nc.vector.tensor_scalar(V[:, 0:2], V[:, 0:2], -1, 0, Op.mult, Op.add)
            nc.vector.tensor_scalar(V[:, 2:4], V[:, 2:4], 1, 0, Op.subtract, Op.add)
            BOF = smallp.tile([P, 4], f32)
            fix = smallp.tile([P, 4], f32)
            nc.vector.tensor_copy(BOF[:], V[:])
            # empty segments (in f32, so the sums round exactly to +/-2^31):
            # mins 32767 -> 2147483648.0, maxes -1 -> -2147483648.0
            nc.vector.tensor_scalar(fix[:, 0:2], BOF[:, 0:2], 32767.0, 2147450880.0,
                                    Op.is_equal, Op.mult)
            nc.vector.tensor_scalar(fix[:, 2:4], BOF[:, 2:4], -1.0, -2147483647.0,
                                    Op.is_equal, Op.mult)
            nc.vector.tensor_tensor(BOF[:], BOF[:], fix[:], Op.add)

            # boxes[f, n, k] <- BOF[n*TL + f, k]
            nc.sync.dma_start(boxes_out[:].transpose([1, 0, 2]), BOF[:])

    nc.finalize()
    if split_waits:
        _split_excess_waits(nc, mybir)
    return nc, tables


def _split_excess_waits(nc, mybir):
    """Hoist extra sem waits onto preceding NoOps.

    This walrus build rejects instructions carrying more sync-wait
    conditions than their ISA encoding holds (1 for TPB_CTRL ops and for
    Pool/core_v2 compute ops; 2 elsewhere, conservatively). Semantics are
    identical with the waits split onto dedicated NoOps just before the
    instruction.
    """
    ctrl = {"Drain", "NoOp", "Nop", "EventSemaphore", "AllEngineBarrier"}
    n_split = 0
    for f in nc.m.functions:
        for bb in f.blocks:
            newl = []
            for ins in bb.instructions:
                si = ins.sync_info
                max_waits = 1
                if si and si.on_wait and len(si.on_wait) > max_waits:
                    waits = list(si.on_wait)
                    for j, w in enumerate(waits[max_waits:]):
                        nop = mybir.InstNoOp(
                            name=f"{ins.name}-w{j}", ins=[], outs=[],
                            engine=ins.engine,
                            sync_info=mybir.SyncInfo(on_wait=[w], on_update=[]))
                        newl.append(nop)
                        n_split += 1
                    ins.sync_info = mybir.SyncInfo(on_wait=waits[:max_waits],
                                                   on_update=si.on_update)
                newl.append(ins)
            bb.instructions = newl
    return n_split


def _get_program(TL, H, W, reps=1):
    key = (TL, H, W, reps)
    if key not in _BUILD_CACHE:
        _BUILD_CACHE[key] = _build_program(TL, H, W, reps=reps)
    return _BUILD_CACHE[key]


def kernel(segmentation, num_instances=None, **_ignored):
    from concourse.bass_utils import run_bass_kernel_spmd

    seg = np.asarray(segmentation)
    T, H, W = seg.shape
    assert T % _NCORES == 0
    TL = T // _NCORES
    nc, tables = _get_program(TL, H, W)

    seg = np.ascontiguousarray(seg, dtype=np.int32)
    in_maps = [{"seg": seg[i * TL:(i + 1) * TL], **tables}
               for i in range(_NCORES)]
    res = run_bass_kernel_spmd(nc, in_maps, list(range(_NCORES)))
    out = np.concatenate([res.results[i]["boxes"] for i in range(_NCORES)], axis=0)
    return out.astype(np.float32)



# revision 25
# speedup vs baseline: 223.1497x; 223.1497x over previous
"""Trainium2 Bass kernel for nn_BoxesFromMasks (per-frame segment bounding boxes).

Algorithm (per core, data-parallel over frames):
  For each frame, build per-pixel one-hot bitmasks of the instance id using an
  exponent-bit trick (int ops construct the bit pattern of float 2^k, an ACT
  copy casts float->uint32 which truncates out-of-range ids to 0):
    lo plane: id s in [0,32)  -> bit (31-s)
    hi plane: id s in [32,64) -> bit (s-32)
  Row masks:  OR-reduce each 128-row chunk along the free (column) axis.
  Col masks:  OR-accumulate chunks into a per-column accumulator, then
              DMA-transpose (as uint16) and OR-reduce along rows.
  Extraction: expand mask bits per id with constant tables, select coordinate
              values, min/max reduce, and partition-fold to one partition.

Scheduling: all frames' chunk work is emitted first; the row-side extraction
is emitted before the per-frame column tails so the DVE keeps busy while the
transpose DMAs land; column extraction follows the tails.
"""

import numpy as np

_T, _H, _W, _N = 16, 1024, 2048, 64
_NCORES = 8

_BUILD_CACHE = {}


def _build_program(TL, H, W, split_waits=True, reps=1, dbg=False):
    from contextlib import ExitStack

    import concourse.bass as bass
    import concourse.tile as tile
    import concourse.mybir as mybir
    from concourse.alu_op_type import AluOpType as Op

    f32 = mybir.dt.float32
    i32 = mybir.dt.int32
    u32 = mybir.dt.uint32
    u16 = mybir.dt.uint16
    Copy = mybir.ActivationFunctionType.Copy
    X = mybir.AxisListType.X

    P = 128
    CH = H // P                   # row chunks per frame
    UC = 2 * W                    # u16 columns per plane
    KT = 8 if UC % (128 * 8) == 0 else UC // 128   # transpose DMA splits
    SPLIT = UC // KT              # u16 cols per transpose call
    MPER = SPLIT // 128           # mid-dim blocks per call
    B = UC // 128                 # total transposed blocks
    BIG = 0x7FFF                  # absent sentinel (fits i16, fp32-exact)
    BIG16 = 0x7FFF

    # ---- constant tables ----
    pp = np.arange(P)
    yv = (np.arange(CH)[None, :] * P + pp[:, None]).astype(np.int64)    # [P, CH]
    bb = np.arange(B)
    xv = ((SPLIT // 2) * (bb[None, :] // MPER) + 64 * (bb[None, :] % MPER)
          + (pp[:, None] % 64)).astype(np.int64)                        # [P, B]
    # select-value scalars, fp32 (tensor_scalar AP scalars must be fp32;
    # every value is < 2^15 so fp32 arithmetic on them is exact)
    # column value per un-parity-sorted transposed partition p and block b:
    # u16col = SPLIT*(b//MPER) + 128*(b%MPER) + p, u32 col = u16col >> 1
    xv4 = ((SPLIT * (bb[None, :] // MPER) + 128 * (bb[None, :] % MPER)
            + pp[:, None]) >> 1).astype(np.int64)                       # [P, B]
    # parity gating: slot (pl, sp) reads halfword h=bit>>4; partition p holds
    # halfword p&1 -> wrong-parity entries select 0 (neutral for min and max)
    spv = np.arange(32)
    bit_lo, bit_hi = 31 - spv, spv                                      # [32]
    hreq = np.stack([bit_lo >> 4, bit_hi >> 4], axis=0)                 # [2, 32]
    pargate = (hreq[None, :, :] == (pp[:, None, None] & 1))             # [P, 2, 32]
    xmB4 = np.where(pargate[:, :, :, None], (xv4 - BIG16)[:, None, None, :], 0)
    xp14 = np.where(pargate[:, :, :, None], (xv4 + 1)[:, None, None, :], 0)
    tables = {
        "ymB": (yv - BIG).astype(np.float32),
        "yp1": (yv + 1).astype(np.float32),
        "xmB4": xmB4.astype(np.int16),      # [P, 2, 32, B]
        "xp14": xp14.astype(np.int16),
    }

    nc = bass.Bass()
    seg_in = nc.dram_tensor("seg", [TL, H, W], i32, kind="ExternalInput")
    boxes_out = nc.dram_tensor("boxes", [TL, 64, 4], f32, kind="ExternalOutput")

    i16 = mybir.dt.int16
    d_ymB = nc.dram_tensor("ymB", [P, CH], f32, kind="ExternalInput")
    d_yp1 = nc.dram_tensor("yp1", [P, CH], f32, kind="ExternalInput")
    d_xmB4 = nc.dram_tensor("xmB4", [P, 2, 32, B], i16, kind="ExternalInput")
    d_xp14 = nc.dram_tensor("xp14", [P, 2, 32, B], i16, kind="ExternalInput")

    if dbg:
        dbg_rmask = nc.dram_tensor("dbg_rmask", [P, TL, 2, CH], u32,
                                   kind="ExternalOutput")
        dbg_cmask = nc.dram_tensor("dbg_cmask", [P, TL, 2, B], u16,
                                   kind="ExternalOutput")

    with tile.TileContext(nc) as tc, ExitStack() as ctx:
        constp = ctx.enter_context(tc.tile_pool(name="consts", bufs=1))
        segp = ctx.enter_context(tc.tile_pool(name="segp", bufs=3))
        ep = ctx.enter_context(tc.tile_pool(name="ep", bufs=3))
        accp = ctx.enter_context(tc.tile_pool(name="accp", bufs=2))
        accTp = ctx.enter_context(tc.tile_pool(name="accTp", bufs=2))
        maskp = ctx.enter_context(tc.tile_pool(name="maskp", bufs=1))
        xp = ctx.enter_context(tc.tile_pool(name="xp", bufs=2))
        trp = ctx.enter_context(tc.tile_pool(name="trp", bufs=2))
        smallp = ctx.enter_context(tc.tile_pool(name="smallp", bufs=1))

        # const tiles declared here, DMAs issued after the first chunk's
        # loads (they are only needed by the extraction phase)
        c_ymB = constp.tile([P, CH], f32)
        c_yp1 = constp.tile([P, CH], f32)
        c_xmB4 = constp.tile([P, 2, 32, B], i16)
        c_xp14 = constp.tile([P, 2, 32, B], i16)

        def load_consts():
            nc.sync.dma_start(c_ymB[:], d_ymB[:])
            nc.sync.dma_start(c_yp1[:], d_yp1[:])
            nc.sync.dma_start(c_xmB4[:], d_xmB4[:])
            nc.sync.dma_start(c_xp14[:], d_xp14[:])

        # body repeated `reps` times (identical output; used for wall-clock
        # device-time measurement: (wall(R) - wall(1)) / (R - 1))
        for _rep in range(reps):
            rmask16 = maskp.tile([P, TL, 2, CH, 16], u32, tag="rmask16")
            # rootc[p, f, pl, b]: column masks at transposed (parity-
            # interleaved) partition order; partition p holds halfword p&1
            rootc = maskp.tile([P, TL, 2, B], u16, tag="rootc")

            # ======== phase A: chunk work, then transposes, per frame ======
            accs, accTs = [], []
            pending = []
            for f in range(TL):
                acc = accp.tile([P, 2, W], u32)
                accs.append(acc)
                prev_u = None
                for c in range(CH):
                    first = (f == 0 and c == 0)
                    s = segp.tile([P, W], i32)
                    e = ep.tile([P, 2, W], i32)
                    u = e[:].bitcast(u32)  # in-place cast target
                    # lo: bitpattern of 2^(31-s) = (158-s)<<23 ;
                    # hi: 2^(s-32) = (s+95)<<23
                    if first:
                        # ramp: load/generate in two column halves so the DVE
                        # starts ~6us sooner
                        for h in range(2):
                            cl = slice(1024 * h, 1024 * (h + 1))
                            for k in range(4):
                                nc.sync.dma_start(
                                    s[32 * k:32 * (k + 1), cl],
                                    seg_in[f, c * P + 32 * k:c * P + 32 * (k + 1), cl])
                            nc.scalar.activation(e[:, 0, cl], s[:, cl], Copy,
                                                 bias=1325400064.0, scale=-8388608.0)
                            nc.gpsimd.tensor_scalar(e[:, 1, cl], s[:, cl],
                                                    8388608, 796917760,
                                                    Op.mult, Op.add)
                            nc.scalar.activation(u[:, 0, cl], e[:, 0, cl].bitcast(f32),
                                                 Copy)
                            nc.gpsimd.tensor_copy(u[:, 1, cl], e[:, 1, cl].bitcast(f32))
                            if h == 0:
                                load_consts()
                    else:
                        for k in range(4):
                            nc.sync.dma_start(
                                s[32 * k:32 * (k + 1), :],
                                seg_in[f, c * P + 32 * k:c * P + 32 * (k + 1), :])
                        nc.scalar.activation(e[:, 0, :], s[:], Copy,
                                             bias=1325400064.0, scale=-8388608.0)
                        nc.gpsimd.tensor_scalar(e[:, 1, :], s[:], 8388608, 796917760,
                                                Op.mult, Op.add)
                        nc.scalar.activation(u[:, 0, :], e[:, 0, :].bitcast(f32), Copy)
                        nc.gpsimd.tensor_copy(u[:, 1, :], e[:, 1, :].bitcast(f32))

                    # row-mask OR-tree into scratch (u stays intact so the
                    # column accumulate is dependency-independent). DVE ops
                    # are emitted in a rotated order -- the previous chunk's
                    # two deepest tree levels (pending) interleave with this
                    # chunk's shallow levels -- so almost every op's input
                    # dependency is >= 2 ops back and the per-dependency
                    # sem delay is absorbed.
                    tr = trp.tile([P, 2, W // 2], u32)
                    base = tr[:]

                    def lvl(dst, lo, hi):
                        return lambda: nc.vector.tensor_tensor(dst, lo, hi,
                                                               Op.bitwise_or)

                    if first:
                        # per-column-half trees so the first half's work can
                        # start before the second half has even loaded
                        for h in range(2):
                            hb, ub = 512 * h, 1024 * h
                            lvl(base[:, :, hb:hb + 512], u[:, :, ub:ub + 512],
                                u[:, :, ub + 512:ub + 1024])()
                            w = 256
                            while w >= 16:
                                lvl(base[:, :, hb:hb + w], base[:, :, hb:hb + w],
                                    base[:, :, hb + w:hb + 2 * w])()
                                w //= 2
                        lvl(rmask16[:, f, :, c, :], base[:, :, 0:16],
                            base[:, :, 512:528])()
                        prev_u = u
                        continue

                    ops = [
                        lvl(base[:, :, 0:1024], u[:, :, 0:1024], u[:, :, 1024:2048]),
                        lvl(base[:, :, 0:512], base[:, :, 0:512], base[:, :, 512:1024]),
                        lvl(base[:, :, 0:256], base[:, :, 0:256], base[:, :, 256:512]),
                        lvl(base[:, :, 0:128], base[:, :, 0:128], base[:, :, 128:256]),
                        lvl(base[:, :, 0:64], base[:, :, 0:64], base[:, :, 64:128]),
                        lvl(base[:, :, 0:32], base[:, :, 0:32], base[:, :, 32:64]),
                        lvl(rmask16[:, f, :, c, :], base[:, :, 0:16],
                            base[:, :, 16:32]),
                    ]
                    last = (c == CH - 1)
                    if c == 0:
                        prev_u = u
                        accop = None
                    elif last:
                        # final accumulate split by column quarters so the
                        # transposes (gated on the acc) can start per-quarter
                        accop = None
                        accqs = [lvl(acc[:, :, 512 * q:512 * (q + 1)],
                                     u[:, :, 512 * q:512 * (q + 1)],
                                     acc[:, :, 512 * q:512 * (q + 1)])
                                 for q in range(4)]
                    elif c == 1:
                        accop = lvl(acc[:], u, prev_u)
                    else:
                        accop = lvl(acc[:], u, acc[:])

                    ops[0]()                                   # L1
                    if pending:
                        pending[0]()                           # L6 of c-1
                    ops[1]()                                   # L2
                    if pending:
                        pending[1]()                           # L7 of c-1
                    ops[2]()                                   # L3
                    if last:
                        for q in range(4):
                            accqs[q]()
                    elif accop:
                        accop()
                    ops[3]()                                   # L4
                    ops[4]()                                   # L5
                    pending = ops[5:7]

                # flush the final chunk's deep levels before the frame tail
                for op in pending:
                    op()
                pending = []

                # launch this frame's column transposes early (SP stream)
                accT = accTp.tile([P, 2, B, 128], u16)
                accTs.append(accT)
                for pl in range(2):
                    a16 = acc[:, pl, :].bitcast(u16)   # [P, UC]
                    for k in range(KT):
                        nc.sync.dma_start(accT[:, pl, k * MPER:(k + 1) * MPER, :],
                                          a16[:, k * SPLIT:(k + 1) * SPLIT],
                                          transpose=True)

            # ========== row-side extraction (overlaps transposes) ==========
            assert 2 * 32 * TL == 128  # per-stat slot block == transpose col set

            # fold rowmask leftovers [.., 16] -> [.., 1]
            rmf = rmask16[:].rearrange("p a b c w -> p (a b c) w")
            w = 8
            while w >= 1:
                nc.vector.tensor_tensor(rmf[:, :, 0:w], rmf[:, :, 0:w],
                                        rmf[:, :, w:2 * w], Op.bitwise_or)
                w //= 2

            if dbg:
                nc.sync.dma_start(dbg_rmask[:], rmask16[:, :, :, :, 0])

            # E = (mask >> bit) & 1  (one op per id slot; int immediates)
            E32 = xp.tile([P, 2, 32, TL, CH], i32, tag="xE")
            for pl in range(2):
                rm_v = rmask16[:, :, pl, :, 0]            # [P, TL, CH]
                for sp in range(32):
                    bit = (31 - sp) if pl == 0 else sp
                    nc.vector.tensor_scalar(
                        E32[:, pl, sp], rm_v.bitcast(i32), bit, 1,
                        Op.logical_shift_right, Op.bitwise_and)

            E16 = xp.tile([P, 2, 32, TL, B], i16, tag="xE")

            # row selects on ACT (overlap the accT trees below):
            # cmin = E*(v-BIG) + BIG in {v, BIG}; cmax = E*(v+1) in {v+1, 0}
            cmin32 = xp.tile([P, 2, 32, TL, CH], i32, tag="xc")
            cmax32 = xp.tile([P, 2, 32, TL, CH], i32, tag="xc")
            for c in range(CH):
                nc.scalar.activation(
                    cmin32[:, :, :, :, c], E32[:, :, :, :, c], Copy,
                    scale=c_ymB[:, c].unsqueeze(1), bias=float(BIG))
                nc.scalar.activation(
                    cmax32[:, :, :, :, c], E32[:, :, :, :, c], Copy,
                    scale=c_yp1[:, c].unsqueeze(1), bias=0.0)

            # ============== per-frame column tails (DVE trees) =============
            for f in range(TL):
                accT = accTs[f]
                w = 64
                while w >= 2:
                    nc.vector.tensor_tensor(accT[:, :, :, 0:w], accT[:, :, :, 0:w],
                                            accT[:, :, :, w:2 * w], Op.bitwise_or)
                    w //= 2
                # final level lands straight in the shared root buffer
                nc.vector.tensor_tensor(rootc[:, f, :, :], accT[:, :, :, 0],
                                        accT[:, :, :, 1], Op.bitwise_or)

            if dbg:
                nc.sync.dma_start(dbg_cmask[:], rootc[:])

            # row-side min/max via in-place TT trees over the chunk axis
            # (TT consumes 2 inputs/cycle; tensor_reduce only 1)
            w = CH // 2
            while w >= 1:
                nc.vector.tensor_tensor(cmin32[:, :, :, :, 0:w],
                                        cmin32[:, :, :, :, 0:w],
                                        cmin32[:, :, :, :, w:2 * w], Op.min)
                nc.vector.tensor_tensor(cmax32[:, :, :, :, 0:w],
                                        cmax32[:, :, :, :, 0:w],
                                        cmax32[:, :, :, :, w:2 * w], Op.max)
                w //= 2

            # combined signed stat tile: S[p, k, pl, s', f], k: 0=-xmin 1=-ymin
            # 2=xmax+1 3=ymax+1 (max-fold works for all four)
            S = smallp.tile([P, 4, 2, 32, TL], i16)

            def srow(k, dt=None):
                ap = S[:, k].rearrange("p a b f -> p (a b f)")
                return ap.bitcast(dt) if dt is not None else ap

            def root(t, dt):
                return t[:, :, :, :, 0].rearrange("p a b f -> p (a b f)").bitcast(dt)

            # row stats are ready first: write their S rows and launch their
            # partition-fold transposes while the column side still computes
            S2 = S[:].rearrange("p k a b f -> p (k a b f)")   # [128, 512]
            ST = smallp.tile([P, 4, 128], i16)
            nc.vector.tensor_scalar(srow(1), root(cmin32, i32), -1, 0,
                                    Op.mult, Op.add)
            nc.vector.tensor_copy(srow(3), root(cmax32, i32))
            for m in (1, 3):
                nc.sync.dma_start(ST[:, m, :], S2[:, 128 * m:128 * (m + 1)],
                                  transpose=True)

            # ==================== column-side extraction ===================
            # bit-extract on ALL partitions regardless of parity; wrong-parity
            # slots pick 0 from the parity-gated value tables (neutral for
            # both the min and the max trees).
            for pl in range(2):
                for sp in range(32):
                    bit = (31 - sp) if pl == 0 else sp
                    inbit = bit & 15
                    cm_v = rootc[:, :, pl, :]                   # [P, TL, B]
                    nc.vector.tensor_scalar(
                        E16[:, pl, sp], cm_v.bitcast(i16), inbit, 1,
                        Op.logical_shift_right, Op.bitwise_and)

            # value-select via one broadcast TT mult each (replaces the per-b
            # ACT loop): cmin16 = E*(v-BIG) in {v-BIG, 0}; 0 is neutral for
            # min since v-BIG < 0. cmax16 = E*(v+1); 0 neutral for max.
            cmin16 = xp.tile([P, 2, 32, TL, B], i16, tag="xc")
            cmax16 = xp.tile([P, 2, 32, TL, B], i16, tag="xc")
            bshape = [P, 2, 32, TL, B]
            Tmin = c_xmB4[:].unsqueeze(3).broadcast_to(bshape)
            Tmax = c_xp14[:].unsqueeze(3).broadcast_to(bshape)
            nc.vector.tensor_tensor(cmin16[:], E16[:], Tmin, Op.mult)
            nc.vector.tensor_tensor(cmax16[:], E16[:], Tmax, Op.mult)
            w = B // 2
            while w >= 1:
                nc.vector.tensor_tensor(cmin16[:, :, :, :, 0:w],
                                        cmin16[:, :, :, :, 0:w],
                                        cmin16[:, :, :, :, w:2 * w], Op.min)
                nc.vector.tensor_tensor(cmax16[:, :, :, :, 0:w],
                                        cmax16[:, :, :, :, 0:w],
                                        cmax16[:, :, :, :, w:2 * w], Op.max)
                w //= 2

            # srow0 = -xmin = -(m + BIG) where m = tree-min of E*(v-BIG)
            nc.vector.tensor_scalar(srow(0), root(cmin16, i16), -1, -BIG16,
                                    Op.mult, Op.add)
            nc.vector.tensor_copy(srow(2), root(cmax16, i16))
            for m in (0, 2):
                nc.sync.dma_start(ST[:, m, :], S2[:, 128 * m:128 * (m + 1)],
                                  transpose=True)
            SR = smallp.tile([P, 4], i16)
            nc.vector.tensor_reduce(SR[:], ST[:], axis=X, op=Op.max)

            # finalize: V[p, k] with p = (pl*32+s')*TL + f
            V = smallp.tile([P, 4], i32)
            nc.vector.tensor_copy(V[:], SR[:])
            nc.vector.tensor_scalar(V[:, 0:2], V[:, 0:2], -1, 0, Op.mult, Op.add)
            nc.vector.tensor_scalar(V[:, 2:4], V[:, 2:4], 1, 0, Op.subtract, Op.add)
            BOF = smallp.tile([P, 4], f32)
            fix = smallp.tile([P, 4], f32)
            nc.vector.tensor_copy(BOF[:], V[:])
            # empty segments (in f32, so the sums round exactly to +/-2^31):
            # mins 32767 -> 2147483648.0, maxes -1 -> -2147483648.0
            nc.vector.tensor_scalar(fix[:, 0:2], BOF[:, 0:2], 32767.0, 2147450880.0,
                                    Op.is_equal, Op.mult)
            nc.vector.tensor_scalar(fix[:, 2:4], BOF[:, 2:4], -1.0, -2147483647.0,
                                    Op.is_equal, Op.mult)
            nc.vector.tensor_tensor(BOF[:], BOF[:], fix[:], Op.add)

            # boxes[f, n, k] <- BOF[n*TL + f, k]
            nc.sync.dma_start(boxes_out[:].transpose([1, 0, 2]), BOF[:])

    nc.finalize()
    if split_waits:
        _split_excess_waits(nc, mybir)
    return nc, tables


def _split_excess_waits(nc, mybir):
    """Hoist extra sem waits onto preceding NoOps.

    This walrus build rejects instructions carrying more sync-wait
    conditions than their ISA encoding holds (1 for TPB_CTRL ops and for
    Pool/core_v2 compute ops; 2 elsewhere, conservatively). Semantics are
    identical with the waits split onto dedicated NoOps just before the
    instruction.
    """
    ctrl = {"Drain", "NoOp", "Nop", "EventSemaphore", "AllEngineBarrier"}
    n_split = 0
    for f in nc.m.functions:
        for bb in f.blocks:
            newl = []
            for ins in bb.instructions:
                si = ins.sync_info
                max_waits = 1
                if si and si.on_wait and len(si.on_wait) > max_waits:
                    waits = list(si.on_wait)
                    for j, w in enumerate(waits[max_waits:]):
                        nop = mybir.InstNoOp(
                            name=f"{ins.name}-w{j}", ins=[], outs=[],
                            engine=ins.engine,
                            sync_info=mybir.SyncInfo(on_wait=[w], on_update=[]))
                        newl.append(nop)
                        n_split += 1
                    ins.sync_info = mybir.SyncInfo(on_wait=waits[:max_waits],
                                                   on_update=si.on_update)
                newl.append(ins)
            bb.instructions = newl
    return n_split


def _get_program(TL, H, W, reps=1):
    key = (TL, H, W, reps)
    if key not in _BUILD_CACHE:
        _BUILD_CACHE[key] = _build_program(TL, H, W, reps=reps)
    return _BUILD_CACHE[key]


def kernel(segmentation, num_instances=None, **_ignored):
    from concourse.bass_utils import run_bass_kernel_spmd

    seg = np.asarray(segmentation)
    T, H, W = seg.shape
    assert T % _NCORES == 0
    TL = T // _NCORES
    nc, tables = _get_program(TL, H, W)

    seg = np.ascontiguousarray(seg, dtype=np.int32)
    in_maps = [{"seg": seg[i * TL:(i + 1) * TL], **tables}
               for i in range(_NCORES)]
    res = run_bass_kernel_spmd(nc, in_maps, list(range(_NCORES)))
    out = np.concatenate([res.results[i]["boxes"] for i in range(_NCORES)], axis=0)
    return out.astype(np.float32)


# revision 41
# speedup vs baseline: 225.0786x; 1.0086x over previous
"""Trainium2 Bass kernel for nn_BoxesFromMasks (per-frame segment bounding boxes).

Algorithm (per core, data-parallel over frames):
  For each frame, build per-pixel one-hot bitmasks of the instance id using an
  exponent-bit trick (int ops construct the bit pattern of float 2^k, an ACT
  copy casts float->uint32 which truncates out-of-range ids to 0):
    lo plane: id s in [0,32)  -> bit (31-s)
    hi plane: id s in [32,64) -> bit (s-32)
  Row masks:  OR-reduce each 128-row chunk along the free (column) axis.
  Col masks:  OR-accumulate chunks into a per-column accumulator, then
              DMA-transpose (as uint16) and OR-reduce along rows.
  Extraction: expand mask bits per id with constant tables, select coordinate
              values, min/max reduce, and partition-fold to one partition.

Scheduling: all frames' chunk work is emitted first; the row-side extraction
is emitted before the per-frame column tails so the DVE keeps busy while the
transpose DMAs land; column extraction follows the tails.
"""

import numpy as np

_T, _H, _W, _N = 16, 1024, 2048, 64
_NCORES = 8

_BUILD_CACHE = {}


def _build_program(TL, H, W, split_waits=True, reps=1, dbg=False):
    from contextlib import ExitStack

    import concourse.bass as bass
    import concourse.tile as tile
    import concourse.mybir as mybir
    from concourse.alu_op_type import AluOpType as Op

    f32 = mybir.dt.float32
    i32 = mybir.dt.int32
    u32 = mybir.dt.uint32
    u16 = mybir.dt.uint16
    Copy = mybir.ActivationFunctionType.Copy
    X = mybir.AxisListType.X

    P = 128
    CH = H // P                   # row chunks per frame
    UC = 2 * W                    # u16 columns per plane
    KT = 8 if UC % (128 * 8) == 0 else UC // 128   # transpose DMA splits
    SPLIT = UC // KT              # u16 cols per transpose call
    MPER = SPLIT // 128           # mid-dim blocks per call
    B = UC // 128                 # total transposed blocks
    BIG = 0x7FFF                  # absent sentinel (fits i16, fp32-exact)
    BIG16 = 0x7FFF

    # ---- constant tables ----
    pp = np.arange(P)
    yv = (np.arange(CH)[None, :] * P + pp[:, None]).astype(np.int64)    # [P, CH]
    bb = np.arange(B)
    xv = ((SPLIT // 2) * (bb[None, :] // MPER) + 64 * (bb[None, :] % MPER)
          + (pp[:, None] % 64)).astype(np.int64)                        # [P, B]
    # select-value scalars, fp32 (tensor_scalar AP scalars must be fp32;
    # every value is < 2^15 so fp32 arithmetic on them is exact)
    # column value per un-parity-sorted transposed partition p and block b:
    # u16col = SPLIT*(b//MPER) + 128*(b%MPER) + p, u32 col = u16col >> 1
    xv4 = ((SPLIT * (bb[None, :] // MPER) + 128 * (bb[None, :] % MPER)
            + pp[:, None]) >> 1).astype(np.int64)                       # [P, B]
    # parity gating: slot (pl, sp) reads halfword h=bit>>4; partition p holds
    # halfword p&1 -> wrong-parity entries select 0 (neutral for min and max)
    spv = np.arange(32)
    bit_lo, bit_hi = 31 - spv, spv                                      # [32]
    hreq = np.stack([bit_lo >> 4, bit_hi >> 4], axis=0)                 # [2, 32]
    pargate = (hreq[None, :, :] == (pp[:, None, None] & 1))             # [P, 2, 32]
    xmB4 = np.where(pargate[:, :, :, None], (xv4 - BIG16)[:, None, None, :], 0)
    xp14 = np.where(pargate[:, :, :, None], (xv4 + 1)[:, None, None, :], 0)
    tables = {
        "ymB": (yv - BIG).astype(np.float32),
        "yp1": (yv + 1).astype(np.float32),
        "xmB4": xmB4.astype(np.int16),      # [P, 2, 32, B]
        "xp14": xp14.astype(np.int16),
    }

    nc = bass.Bass()
    seg_in = nc.dram_tensor("seg", [TL, H, W], i32, kind="ExternalInput")
    boxes_out = nc.dram_tensor("boxes", [TL, 64, 4], f32, kind="ExternalOutput")

    i16 = mybir.dt.int16
    d_ymB = nc.dram_tensor("ymB", [P, CH], f32, kind="ExternalInput")
    d_yp1 = nc.dram_tensor("yp1", [P, CH], f32, kind="ExternalInput")
    d_xmB4 = nc.dram_tensor("xmB4", [P, 2, 32, B], i16, kind="ExternalInput")
    d_xp14 = nc.dram_tensor("xp14", [P, 2, 32, B], i16, kind="ExternalInput")

    if dbg:
        dbg_rmask = nc.dram_tensor("dbg_rmask", [P, TL, 2, CH], u32,
                                   kind="ExternalOutput")
        dbg_cmask = nc.dram_tensor("dbg_cmask", [P, TL, 2, B], u16,
                                   kind="ExternalOutput")

    with tile.TileContext(nc) as tc, ExitStack() as ctx:
        constp = ctx.enter_context(tc.tile_pool(name="consts", bufs=1))
        segp = ctx.enter_context(tc.tile_pool(name="segp", bufs=2))
        ep = ctx.enter_context(tc.tile_pool(name="ep", bufs=3))
        accp = ctx.enter_context(tc.tile_pool(name="accp", bufs=2))
        accTp = ctx.enter_context(tc.tile_pool(name="accTp", bufs=2))
        maskp = ctx.enter_context(tc.tile_pool(name="maskp", bufs=1))
        xp = ctx.enter_context(tc.tile_pool(name="xp", bufs=2))
        xc16p = ctx.enter_context(tc.tile_pool(name="xc16", bufs=2))
        trp = ctx.enter_context(tc.tile_pool(name="trp", bufs=2))
        smallp = ctx.enter_context(tc.tile_pool(name="smallp", bufs=1))

        # const tiles declared here, DMAs issued after the first chunk's
        # loads (they are only needed by the extraction phase)
        c_ymB = constp.tile([P, CH], f32)
        c_yp1 = constp.tile([P, CH], f32)
        c_xmB4 = constp.tile([P, 2, 32, B], i16)
        c_xp14 = constp.tile([P, 2, 32, B], i16)

        def load_consts():
            nc.sync.dma_start(c_ymB[:], d_ymB[:])
            nc.sync.dma_start(c_yp1[:], d_yp1[:])
            nc.sync.dma_start(c_xmB4[:], d_xmB4[:])
            nc.sync.dma_start(c_xp14[:], d_xp14[:])

        # body repeated `reps` times (identical output; used for wall-clock
        # device-time measurement: (wall(R) - wall(1)) / (R - 1))
        for _rep in range(reps):
            rmask16 = maskp.tile([P, TL, 2, CH, 16], u32, tag="rmask16")
            # rootc[p, f, pl, b]: column masks at transposed (parity-
            # interleaved) partition order; partition p holds halfword p&1
            rootc = maskp.tile([P, TL, 2, B], u16, tag="rootc")
            # ======== phase A: chunk work, then transposes, per frame ======
            accs, accTs = [], []
            pending = []
            for f in range(TL):
                acc = accp.tile([P, 2, W], u32)
                accs.append(acc)
                prev_u = None
                for c in range(CH):
                    first = (f == 0 and c <= 1)
                    s = segp.tile([P, W], i32)
                    e = ep.tile([P, 2, W], i32)
                    u = e[:].bitcast(u32)  # in-place cast target
                    # lo: bitpattern of 2^(31-s) = (158-s)<<23 ;
                    # hi: 2^(s-32) = (s+95)<<23
                    if first:
                        # ramp: load/generate in two column halves so the DVE
                        # starts ~6us sooner
                        for h in range(2):
                            cl = slice(1024 * h, 1024 * (h + 1))
                            for k in range(4):
                                nc.sync.dma_start(
                                    s[32 * k:32 * (k + 1), cl],
                                    seg_in[f, c * P + 32 * k:c * P + 32 * (k + 1), cl])
                            nc.scalar.activation(e[:, 0, cl], s[:, cl], Copy,
                                                 bias=1325400064.0, scale=-8388608.0)
                            nc.gpsimd.tensor_scalar(e[:, 1, cl], s[:, cl],
                                                    8388608, 796917760,
                                                    Op.mult, Op.add)
                            nc.scalar.activation(u[:, 0, cl], e[:, 0, cl].bitcast(f32),
                                                 Copy)
                            nc.gpsimd.tensor_copy(u[:, 1, cl], e[:, 1, cl].bitcast(f32))
                            if c == 0 and h == 0:
                                load_consts()
                    else:
                        for k in range(4):
                            nc.sync.dma_start(
                                s[32 * k:32 * (k + 1), :],
                                seg_in[f, c * P + 32 * k:c * P + 32 * (k + 1), :])
                        nc.scalar.activation(e[:, 0, :], s[:], Copy,
                                             bias=1325400064.0, scale=-8388608.0)
                        nc.gpsimd.tensor_scalar(e[:, 1, :], s[:], 8388608, 796917760,
                                                Op.mult, Op.add)
                        nc.scalar.activation(u[:, 0, :], e[:, 0, :].bitcast(f32), Copy)
                        nc.gpsimd.tensor_copy(u[:, 1, :], e[:, 1, :].bitcast(f32))

                    # row-mask OR-tree into scratch (u stays intact so the
                    # column accumulate is dependency-independent). DVE ops
                    # are emitted in a rotated order -- the previous chunk's
                    # two deepest tree levels (pending) interleave with this
                    # chunk's shallow levels -- so almost every op's input
                    # dependency is >= 2 ops back and the per-dependency
                    # sem delay is absorbed.
                    tr = trp.tile([P, 2, W // 2], u32)
                    base = tr[:]

                    def lvl(dst, lo, hi):
                        return lambda: nc.vector.tensor_tensor(dst, lo, hi,
                                                               Op.bitwise_or)

                    if first:
                        # per-column-half trees (and, for c==1, half-width
                        # accumulates) so each half's work starts as soon as
                        # its cast lands during the pipeline ramp
                        for h in range(2):
                            hb, ub = 512 * h, 1024 * h
                            if c == 1:
                                cl = slice(1024 * h, 1024 * (h + 1))
                                lvl(acc[:, :, cl], u[:, :, cl],
                                    prev_u[:, :, cl])()
                            elif c > 1:
                                cl = slice(1024 * h, 1024 * (h + 1))
                                lvl(acc[:, :, cl], u[:, :, cl],
                                    acc[:, :, cl])()
                            lvl(base[:, :, hb:hb + 512], u[:, :, ub:ub + 512],
                                u[:, :, ub + 512:ub + 1024])()
                            w = 256
                            while w >= 16:
                                lvl(base[:, :, hb:hb + w], base[:, :, hb:hb + w],
                                    base[:, :, hb + w:hb + 2 * w])()
                                w //= 2
                        lvl(rmask16[:, f, :, c, :], base[:, :, 0:16],
                            base[:, :, 512:528])()
                        prev_u = u
                        continue

                    ops = [
                        lvl(base[:, :, 0:1024], u[:, :, 0:1024], u[:, :, 1024:2048]),
                        lvl(base[:, :, 0:512], base[:, :, 0:512], base[:, :, 512:1024]),
                        lvl(base[:, :, 0:256], base[:, :, 0:256], base[:, :, 256:512]),
                        lvl(base[:, :, 0:128], base[:, :, 0:128], base[:, :, 128:256]),
                        lvl(base[:, :, 0:64], base[:, :, 0:64], base[:, :, 64:128]),
                        lvl(base[:, :, 0:32], base[:, :, 0:32], base[:, :, 32:64]),
                        lvl(rmask16[:, f, :, c, :], base[:, :, 0:16],
                            base[:, :, 16:32]),
                    ]
                    last = (c == CH - 1)
                    if c == 0:
                        prev_u = u
                        accop = None
                    elif last:
                        # final accumulate split by column quarters so the
                        # transposes (gated on the acc) can start per-quarter
                        accop = None
                        accqs = [lvl(acc[:, :, 512 * q:512 * (q + 1)],
                                     u[:, :, 512 * q:512 * (q + 1)],
                                     acc[:, :, 512 * q:512 * (q + 1)])
                                 for q in range(4)]
                    elif c == 1:
                        accop = lvl(acc[:], u, prev_u)
                    else:
                        accop = lvl(acc[:], u, acc[:])

                    ops[0]()                                   # L1
                    if pending:
                        pending[0]()                           # L6 of c-1
                    ops[1]()                                   # L2
                    if pending:
                        pending[1]()                           # L7 of c-1
                    ops[2]()                                   # L3
                    if last:
                        for q in range(4):
                            accqs[q]()
                    elif accop:
                        accop()
                    ops[3]()                                   # L4
                    ops[4]()                                   # L5
                    pending = ops[5:7]

                # flush the final chunk's deep levels before the frame tail
                for op in pending:
                    op()
                pending = []

                # launch this frame's column transposes early (SP stream)
                accT = accTp.tile([P, 2, B, 128], u16)
                accTs.append(accT)
                for pl in range(2):
                    a16 = acc[:, pl, :].bitcast(u16)   # [P, UC]
                    for k in range(KT):
                        nc.sync.dma_start(accT[:, pl, k * MPER:(k + 1) * MPER, :],
                                          a16[:, k * SPLIT:(k + 1) * SPLIT],
                                          transpose=True)

            # ========== row-side extraction (overlaps transposes) ==========
            assert 2 * 32 * TL == 128  # per-stat slot block == transpose col set

            # fold rowmask leftovers [.., 16] -> [.., 1]
            rmf = rmask16[:].rearrange("p a b c w -> p (a b c) w")
            w = 8
            while w >= 1:
                nc.vector.tensor_tensor(rmf[:, :, 0:w], rmf[:, :, 0:w],
                                        rmf[:, :, w:2 * w], Op.bitwise_or)
                w //= 2

            if dbg:
                nc.sync.dma_start(dbg_rmask[:], rmask16[:, :, :, :, 0])

            # E = (mask >> bit) & 1  (one op per id slot; int immediates)
            E32 = xp.tile([P, 2, 32, TL, CH], i32, tag="xE")
            for pl in range(2):
                rm_v = rmask16[:, :, pl, :, 0]            # [P, TL, CH]
                for sp in range(32):
                    bit = (31 - sp) if pl == 0 else sp
                    nc.vector.tensor_scalar(
                        E32[:, pl, sp], rm_v.bitcast(i32), bit, 1,
                        Op.logical_shift_right, Op.bitwise_and)

            # row selects on ACT (overlap the accT trees below):
            # cmin = E*(v-BIG) + BIG in {v, BIG}; cmax = E*(v+1) in {v+1, 0}
            cmin32 = xp.tile([P, 2, 32, TL, CH], i32, tag="xc")
            cmax32 = xp.tile([P, 2, 32, TL, CH], i32, tag="xc")
            for c in range(CH):
                nc.scalar.activation(
                    cmin32[:, :, :, :, c], E32[:, :, :, :, c], Copy,
                    scale=c_ymB[:, c].unsqueeze(1), bias=float(BIG))
                nc.scalar.activation(
                    cmax32[:, :, :, :, c], E32[:, :, :, :, c], Copy,
                    scale=c_yp1[:, c].unsqueeze(1), bias=0.0)

            # ============== per-frame column tails (DVE trees) =============
            for f in range(TL):
                accT = accTs[f]
                w = 64
                while w >= 2:
                    nc.vector.tensor_tensor(accT[:, :, :, 0:w], accT[:, :, :, 0:w],
                                            accT[:, :, :, w:2 * w], Op.bitwise_or)
                    w //= 2
                # final level lands straight in the shared root buffer
                nc.vector.tensor_tensor(rootc[:, f, :, :], accT[:, :, :, 0],
                                        accT[:, :, :, 1], Op.bitwise_or)

            if dbg:
                nc.sync.dma_start(dbg_cmask[:], rootc[:])

            # row-side min/max via in-place TT trees over the chunk axis
            # (TT consumes 2 inputs/cycle; tensor_reduce only 1)
            w = CH // 2
            while w >= 1:
                nc.vector.tensor_tensor(cmin32[:, :, :, :, 0:w],
                                        cmin32[:, :, :, :, 0:w],
                                        cmin32[:, :, :, :, w:2 * w], Op.min)
                nc.vector.tensor_tensor(cmax32[:, :, :, :, 0:w],
                                        cmax32[:, :, :, :, 0:w],
                                        cmax32[:, :, :, :, w:2 * w], Op.max)
                w //= 2

            # combined signed stat tile: S[p, k, pl, s', f], k: 0=-xmin
            # 1=-ymin 2=xmax+1 3=ymax+1 (max-fold works for all four)
            S = smallp.tile([P, 4, 2, 32, TL], i16)
            ST = smallp.tile([P, 4, 128], i16)
            S2 = S[:].rearrange("p k a b f -> p (k a b f)")   # [128, 512]

            def srow(k, dt=None):
                ap = S[:, k].rearrange("p a b f -> p (a b f)")
                return ap.bitcast(dt) if dt is not None else ap

            def root(t, dt):
                return t[:, :, :, :, 0].rearrange("p a b f -> p (a b f)").bitcast(dt)

            # row stats are ready first: write their S rows and launch their
            # partition-fold transposes while the column side still computes
            nc.vector.tensor_scalar(srow(1), root(cmin32, i32), -1, 0,
                                    Op.mult, Op.add)
            nc.vector.tensor_copy(srow(3), root(cmax32, i32))
            for m in (1, 3):
                nc.sync.dma_start(ST[:, m, :], S2[:, 128 * m:128 * (m + 1)],
                                  transpose=True)

            # ==================== column-side extraction ===================
            # bit-extract on ALL partitions regardless of parity; wrong-parity
            # slots pick 0 from the parity-gated value tables (neutral for
            # both the min and the max trees).
            E16 = xp.tile([P, 2, 32, TL, B], i16, tag="xE")
            for pl in range(2):
                for sp in range(32):
                    bit = (31 - sp) if pl == 0 else sp
                    nc.vector.tensor_scalar(
                        E16[:, pl, sp], rootc[:, :, pl, :].bitcast(i16),
                        bit & 15, 1, Op.logical_shift_right, Op.bitwise_and)

            # value-select via one broadcast TT mult each: cmin16 = E*(v-BIG)
            # in {v-BIG, 0}; 0 neutral for min since v-BIG < 0. cmax16 =
            # E*(v+1); 0 neutral for max.
            cmin16 = xp.tile([P, 2, 32, TL, B], i16, tag="xc")
            cmax16 = xp.tile([P, 2, 32, TL, B], i16, tag="xc")
            bshape = [P, 2, 32, TL, B]
            Tmin = c_xmB4[:].unsqueeze(3).broadcast_to(bshape)
            Tmax = c_xp14[:].unsqueeze(3).broadcast_to(bshape)
            nc.vector.tensor_tensor(cmin16[:], E16[:], Tmin, Op.mult)
            nc.vector.tensor_tensor(cmax16[:], E16[:], Tmax, Op.mult)
            w = B // 2
            while w >= 1:
                nc.vector.tensor_tensor(cmin16[:, :, :, :, 0:w],
                                        cmin16[:, :, :, :, 0:w],
                                        cmin16[:, :, :, :, w:2 * w], Op.min)
                nc.vector.tensor_tensor(cmax16[:, :, :, :, 0:w],
                                        cmax16[:, :, :, :, 0:w],
                                        cmax16[:, :, :, :, w:2 * w], Op.max)
                w //= 2

            # srow0 = -xmin = -(m + BIG) where m = tree-min of E*(v-BIG)
            nc.vector.tensor_scalar(srow(0), root(cmin16, i16), -1, -BIG16,
                                    Op.mult, Op.add)
            nc.vector.tensor_copy(srow(2), root(cmax16, i16))
            for m in (0, 2):
                nc.sync.dma_start(ST[:, m, :], S2[:, 128 * m:128 * (m + 1)],
                                  transpose=True)
            SR = smallp.tile([P, 4], i16)
            nc.vector.tensor_reduce(SR[:], ST[:], axis=X, op=Op.max)

            # finalize: V[p, k] with p = (pl*32+s')*TL + f
            V = smallp.tile([P, 4], i32)
            nc.vector.tensor_copy(V[:], SR[:])
            nc.vector.tensor_scalar(V[:, 0:2], V[:, 0:2], -1, 0, Op.mult, Op.add)
            nc.vector.tensor_scalar(V[:, 2:4], V[:, 2:4], 1, 0, Op.subtract, Op.add)
            BOF = smallp.tile([P, 4], f32)
            fix = smallp.tile([P, 4], f32)
            nc.vector.tensor_copy(BOF[:], V[:])
            # empty segments (in f32, so the sums round exactly to +/-2^31):
            # mins 32767 -> 2147483648.0, maxes -1 -> -2147483648.0
            nc.vector.tensor_scalar(fix[:, 0:2], BOF[:, 0:2], 32767.0, 2147450880.0,
                                    Op.is_equal, Op.mult)
            nc.vector.tensor_scalar(fix[:, 2:4], BOF[:, 2:4], -1.0, -2147483647.0,
                                    Op.is_equal, Op.mult)
            nc.vector.tensor_tensor(BOF[:], BOF[:], fix[:], Op.add)

            # boxes[f, n, k] <- BOF[n*TL + f, k]
            nc.sync.dma_start(boxes_out[:].transpose([1, 0, 2]), BOF[:])

    nc.finalize()
    if split_waits:
        _split_excess_waits(nc, mybir)
    return nc, tables


def _split_excess_waits(nc, mybir):
    """Hoist extra sem waits onto preceding NoOps.

    This walrus build rejects instructions carrying more sync-wait
    conditions than their ISA encoding holds (1 for TPB_CTRL ops and for
    Pool/core_v2 compute ops; 2 elsewhere, conservatively). Semantics are
    identical with the waits split onto dedicated NoOps just before the
    instruction.
    """
    ctrl = {"Drain", "NoOp", "Nop", "EventSemaphore", "AllEngineBarrier"}
    n_split = 0
    for f in nc.m.functions:
        for bb in f.blocks:
            newl = []
            for ins in bb.instructions:
                si = ins.sync_info
                max_waits = 1
                if si and si.on_wait and len(si.on_wait) > max_waits:
                    waits = list(si.on_wait)
                    for j, w in enumerate(waits[max_waits:]):
                        nop = mybir.InstNoOp(
                            name=f"{ins.name}-w{j}", ins=[], outs=[],
                            engine=ins.engine,
                            sync_info=mybir.SyncInfo(on_wait=[w], on_update=[]))
                        newl.append(nop)
                        n_split += 1
                    ins.sync_info = mybir.SyncInfo(on_wait=waits[:max_waits],
                                                   on_update=si.on_update)
                newl.append(ins)
            bb.instructions = newl
    return n_split


def _get_program(TL, H, W, reps=1):
    key = (TL, H, W, reps)
    if key not in _BUILD_CACHE:
        _BUILD_CACHE[key] = _build_program(TL, H, W, reps=reps)
    return _BUILD_CACHE[key]


def kernel(segmentation, num_instances=None, **_ignored):
    from concourse.bass_utils import run_bass_kernel_spmd

    seg = np.asarray(segmentation)
    T, H, W = seg.shape
    assert T % _NCORES == 0
    TL = T // _NCORES
    nc, tables = _get_program(TL, H, W)

    seg = np.ascontiguousarray(seg, dtype=np.int32)
    in_maps = [{"seg": seg[i * TL:(i + 1) * TL], **tables}
               for i in range(_NCORES)]
    res = run_bass_kernel_spmd(nc, in_maps, list(range(_NCORES)))
    out = np.concatenate([res.results[i]["boxes"] for i in range(_NCORES)], axis=0)
    return out.astype(np.float32)


# revision 45
# speedup vs baseline: 231.4424x; 1.0283x over previous
"""Trainium2 Bass kernel for nn_BoxesFromMasks (per-frame segment bounding boxes).

Algorithm (per core, data-parallel over frames):
  For each frame, build per-pixel one-hot bitmasks of the instance id using an
  exponent-bit trick (int ops construct the bit pattern of float 2^k, an ACT
  copy casts float->uint32 which truncates out-of-range ids to 0):
    lo plane: id s in [0,32)  -> bit (31-s)
    hi plane: id s in [32,64) -> bit (s-32)
  Row masks:  OR-reduce each 128-row chunk along the free (column) axis.
  Col masks:  OR-accumulate chunks into a per-column accumulator, then
              DMA-transpose (as uint16) and OR-reduce along rows.
  Extraction: expand mask bits per id with constant tables, select coordinate
              values, min/max reduce, and partition-fold to one partition.

Scheduling: all frames' chunk work is emitted first; the row-side extraction
is emitted before the per-frame column tails so the DVE keeps busy while the
transpose DMAs land; column extraction follows the tails.
"""

import numpy as np

_T, _H, _W, _N = 16, 1024, 2048, 64
_NCORES = 8

_BUILD_CACHE = {}


def _build_program(TL, H, W, split_waits=True, reps=1, dbg=False):
    from contextlib import ExitStack

    import concourse.bass as bass
    import concourse.tile as tile
    import concourse.mybir as mybir
    from concourse.alu_op_type import AluOpType as Op

    f32 = mybir.dt.float32
    i32 = mybir.dt.int32
    u32 = mybir.dt.uint32
    u16 = mybir.dt.uint16
    Copy = mybir.ActivationFunctionType.Copy
    X = mybir.AxisListType.X

    P = 128
    CH = H // P                   # row chunks per frame
    UC = 2 * W                    # u16 columns per plane
    KT = 2 if UC % (128 * 2) == 0 else UC // 128   # transpose DMA splits
    SPLIT = UC // KT              # u16 cols per transpose call
    MPER = SPLIT // 128           # mid-dim blocks per call
    B = UC // 128                 # total transposed blocks
    BIG = 0x7FFF                  # absent sentinel (fits i16, fp32-exact)
    BIG16 = 0x7FFF

    # ---- constant tables ----
    pp = np.arange(P)
    yv = (np.arange(CH)[None, :] * P + pp[:, None]).astype(np.int64)    # [P, CH]
    bb = np.arange(B)
    xv = ((SPLIT // 2) * (bb[None, :] // MPER) + 64 * (bb[None, :] % MPER)
          + (pp[:, None] % 64)).astype(np.int64)                        # [P, B]
    # select-value scalars, fp32 (tensor_scalar AP scalars must be fp32;
    # every value is < 2^15 so fp32 arithmetic on them is exact)
    # column value per un-parity-sorted transposed partition p and block b:
    # u16col = SPLIT*(b//MPER) + 128*(b%MPER) + p, u32 col = u16col >> 1
    xv4 = ((SPLIT * (bb[None, :] // MPER) + 128 * (bb[None, :] % MPER)
            + pp[:, None]) >> 1).astype(np.int64)                       # [P, B]
    # parity gating: slot (pl, sp) reads halfword h=bit>>4; partition p holds
    # halfword p&1 -> wrong-parity entries select 0 (neutral for min and max)
    spv = np.arange(32)
    bit_lo, bit_hi = 31 - spv, spv                                      # [32]
    hreq = np.stack([bit_lo >> 4, bit_hi >> 4], axis=0)                 # [2, 32]
    pargate = (hreq[None, :, :] == (pp[:, None, None] & 1))             # [P, 2, 32]
    xmB4 = np.where(pargate[:, :, :, None], (xv4 - BIG16)[:, None, None, :], 0)
    xp14 = np.where(pargate[:, :, :, None], (xv4 + 1)[:, None, None, :], 0)
    tables = {
        "ymB": (yv - BIG).astype(np.float32),
        "yp1": (yv + 1).astype(np.float32),
        "xmB4": xmB4.astype(np.int16),      # [P, 2, 32, B]
        "xp14": xp14.astype(np.int16),
    }

    nc = bass.Bass()
    seg_in = nc.dram_tensor("seg", [TL, H, W], i32, kind="ExternalInput")
    boxes_out = nc.dram_tensor("boxes", [TL, 64, 4], f32, kind="ExternalOutput")

    i16 = mybir.dt.int16
    d_ymB = nc.dram_tensor("ymB", [P, CH], f32, kind="ExternalInput")
    d_yp1 = nc.dram_tensor("yp1", [P, CH], f32, kind="ExternalInput")
    d_xmB4 = nc.dram_tensor("xmB4", [P, 2, 32, B], i16, kind="ExternalInput")
    d_xp14 = nc.dram_tensor("xp14", [P, 2, 32, B], i16, kind="ExternalInput")

    if dbg:
        dbg_rmask = nc.dram_tensor("dbg_rmask", [P, TL, 2, CH], u32,
                                   kind="ExternalOutput")
        dbg_cmask = nc.dram_tensor("dbg_cmask", [P, TL, 2, B], u16,
                                   kind="ExternalOutput")

    with tile.TileContext(nc) as tc, ExitStack() as ctx:
        constp = ctx.enter_context(tc.tile_pool(name="consts", bufs=1))
        segp = ctx.enter_context(tc.tile_pool(name="segp", bufs=2))
        ep = ctx.enter_context(tc.tile_pool(name="ep", bufs=3))
        accp = ctx.enter_context(tc.tile_pool(name="accp", bufs=2))
        accTp = ctx.enter_context(tc.tile_pool(name="accTp", bufs=2))
        maskp = ctx.enter_context(tc.tile_pool(name="maskp", bufs=1))
        xp = ctx.enter_context(tc.tile_pool(name="xp", bufs=2))
        xc16p = ctx.enter_context(tc.tile_pool(name="xc16", bufs=2))
        trp = ctx.enter_context(tc.tile_pool(name="trp", bufs=2))
        smallp = ctx.enter_context(tc.tile_pool(name="smallp", bufs=1))

        # const tiles declared here, DMAs issued after the first chunk's
        # loads (they are only needed by the extraction phase)
        c_ymB = constp.tile([P, CH], f32)
        c_yp1 = constp.tile([P, CH], f32)
        c_xmB4 = constp.tile([P, 2, 32, B], i16)
        c_xp14 = constp.tile([P, 2, 32, B], i16)

        def load_consts():
            nc.sync.dma_start(c_ymB[:], d_ymB[:])
            nc.sync.dma_start(c_yp1[:], d_yp1[:])
            nc.sync.dma_start(c_xmB4[:], d_xmB4[:])
            nc.sync.dma_start(c_xp14[:], d_xp14[:])

        # body repeated `reps` times (identical output; used for wall-clock
        # device-time measurement: (wall(R) - wall(1)) / (R - 1))
        for _rep in range(reps):
            rmask16 = maskp.tile([P, TL, 2, CH, 16], u32, tag="rmask16")
            # rootc[p, f, pl, b]: column masks at transposed (parity-
            # interleaved) partition order; partition p holds halfword p&1
            rootc = maskp.tile([P, TL, 2, B], u16, tag="rootc")
            # ======== phase A: chunk work, then transposes, per frame ======
            accs, accTs = [], []
            pending = []
            for f in range(TL):
                acc = accp.tile([P, 2, W], u32)
                accs.append(acc)
                prev_u = None
                for c in range(CH):
                    first = (f == 0 and c <= 1)
                    s = segp.tile([P, W], i32)
                    e = ep.tile([P, 2, W], i32)
                    u = e[:].bitcast(u32)  # in-place cast target
                    # lo: bitpattern of 2^(31-s) = (158-s)<<23 ;
                    # hi: 2^(s-32) = (s+95)<<23
                    if first:
                        # ramp: load/generate in two column halves so the DVE
                        # starts ~6us sooner
                        for h in range(2):
                            cl = slice(1024 * h, 1024 * (h + 1))
                            for k in range(4):
                                nc.sync.dma_start(
                                    s[32 * k:32 * (k + 1), cl],
                                    seg_in[f, c * P + 32 * k:c * P + 32 * (k + 1), cl])
                            nc.scalar.activation(e[:, 0, cl], s[:, cl], Copy,
                                                 bias=1325400064.0, scale=-8388608.0)
                            nc.gpsimd.tensor_scalar(e[:, 1, cl], s[:, cl],
                                                    8388608, 796917760,
                                                    Op.mult, Op.add)
                            nc.scalar.activation(u[:, 0, cl], e[:, 0, cl].bitcast(f32),
                                                 Copy)
                            nc.gpsimd.tensor_copy(u[:, 1, cl], e[:, 1, cl].bitcast(f32))
                            if c == 0 and h == 0:
                                load_consts()
                    else:
                        for k in range(4):
                            nc.sync.dma_start(
                                s[32 * k:32 * (k + 1), :],
                                seg_in[f, c * P + 32 * k:c * P + 32 * (k + 1), :])
                        nc.scalar.activation(e[:, 0, :], s[:], Copy,
                                             bias=1325400064.0, scale=-8388608.0)
                        nc.gpsimd.tensor_scalar(e[:, 1, :], s[:], 8388608, 796917760,
                                                Op.mult, Op.add)
                        nc.scalar.activation(u[:, 0, :], e[:, 0, :].bitcast(f32), Copy)
                        nc.gpsimd.tensor_copy(u[:, 1, :], e[:, 1, :].bitcast(f32))

                    # row-mask OR-tree into scratch (u stays intact so the
                    # column accumulate is dependency-independent). DVE ops
                    # are emitted in a rotated order -- the previous chunk's
                    # two deepest tree levels (pending) interleave with this
                    # chunk's shallow levels -- so almost every op's input
                    # dependency is >= 2 ops back and the per-dependency
                    # sem delay is absorbed.
                    tr = trp.tile([P, 2, W // 2], u32)
                    base = tr[:]

                    def lvl(dst, lo, hi):
                        return lambda: nc.vector.tensor_tensor(dst, lo, hi,
                                                               Op.bitwise_or)

                    if first:
                        # per-column-half trees (and, for c==1, half-width
                        # accumulates) so each half's work starts as soon as
                        # its cast lands during the pipeline ramp
                        for h in range(2):
                            hb, ub = 512 * h, 1024 * h
                            if c == 1:
                                cl = slice(1024 * h, 1024 * (h + 1))
                                lvl(acc[:, :, cl], u[:, :, cl],
                                    prev_u[:, :, cl])()
                            elif c > 1:
                                cl = slice(1024 * h, 1024 * (h + 1))
                                lvl(acc[:, :, cl], u[:, :, cl],
                                    acc[:, :, cl])()
                            lvl(base[:, :, hb:hb + 512], u[:, :, ub:ub + 512],
                                u[:, :, ub + 512:ub + 1024])()
                            w = 256
                            while w >= 16:
                                lvl(base[:, :, hb:hb + w], base[:, :, hb:hb + w],
                                    base[:, :, hb + w:hb + 2 * w])()
                                w //= 2
                        lvl(rmask16[:, f, :, c, :], base[:, :, 0:16],
                            base[:, :, 512:528])()
                        prev_u = u
                        continue

                    ops = [
                        lvl(base[:, :, 0:1024], u[:, :, 0:1024], u[:, :, 1024:2048]),
                        lvl(base[:, :, 0:512], base[:, :, 0:512], base[:, :, 512:1024]),
                        lvl(base[:, :, 0:256], base[:, :, 0:256], base[:, :, 256:512]),
                        lvl(base[:, :, 0:128], base[:, :, 0:128], base[:, :, 128:256]),
                        lvl(base[:, :, 0:64], base[:, :, 0:64], base[:, :, 64:128]),
                        lvl(base[:, :, 0:32], base[:, :, 0:32], base[:, :, 32:64]),
                        lvl(rmask16[:, f, :, c, :], base[:, :, 0:16],
                            base[:, :, 16:32]),
                    ]
                    last = (c == CH - 1)
                    if c == 0:
                        prev_u = u
                        accop = None
                    elif last:
                        # final accumulate split by column quarters so the
                        # transposes (gated on the acc) can start per-quarter
                        accop = None
                        accqs = [lvl(acc[:, :, 512 * q:512 * (q + 1)],
                                     u[:, :, 512 * q:512 * (q + 1)],
                                     acc[:, :, 512 * q:512 * (q + 1)])
                                 for q in range(4)]
                    elif c == 1:
                        accop = lvl(acc[:], u, prev_u)
                    else:
                        accop = lvl(acc[:], u, acc[:])

                    ops[0]()                                   # L1
                    if pending:
                        pending[0]()                           # L6 of c-1
                    ops[1]()                                   # L2
                    if pending:
                        pending[1]()                           # L7 of c-1
                    ops[2]()                                   # L3
                    if last:
                        for q in range(4):
                            accqs[q]()
                    elif accop:
                        accop()
                    ops[3]()                                   # L4
                    ops[4]()                                   # L5
                    pending = ops[5:7]

                # flush the final chunk's deep levels before the frame tail
                for op in pending:
                    op()
                pending = []

                # launch this frame's column transposes early (SP stream)
                accT = accTp.tile([P, 2, B, 128], u16)
                accTs.append(accT)
                for pl in range(2):
                    a16 = acc[:, pl, :].bitcast(u16)   # [P, UC]
                    for k in range(KT):
                        nc.sync.dma_start(accT[:, pl, k * MPER:(k + 1) * MPER, :],
                                          a16[:, k * SPLIT:(k + 1) * SPLIT],
                                          transpose=True)

            # ========== row-side extraction (overlaps transposes) ==========
            assert 2 * 32 * TL == 128  # per-stat slot block == transpose col set

            # fold rowmask leftovers [.., 16] -> [.., 1]
            rmf = rmask16[:].rearrange("p a b c w -> p (a b c) w")
            w = 8
            while w >= 1:
                nc.vector.tensor_tensor(rmf[:, :, 0:w], rmf[:, :, 0:w],
                                        rmf[:, :, w:2 * w], Op.bitwise_or)
                w //= 2

            if dbg:
                nc.sync.dma_start(dbg_rmask[:], rmask16[:, :, :, :, 0])

            # E = (mask >> bit) & 1  (one op per id slot; int immediates)
            E32 = xp.tile([P, 2, 32, TL, CH], i32, tag="xE")
            for pl in range(2):
                rm_v = rmask16[:, :, pl, :, 0]            # [P, TL, CH]
                for sp in range(32):
                    bit = (31 - sp) if pl == 0 else sp
                    nc.vector.tensor_scalar(
                        E32[:, pl, sp], rm_v.bitcast(i32), bit, 1,
                        Op.logical_shift_right, Op.bitwise_and)

            # row selects on ACT (overlap the accT trees below):
            # cmin = E*(v-BIG) + BIG in {v, BIG}; cmax = E*(v+1) in {v+1, 0}
            cmin32 = xp.tile([P, 2, 32, TL, CH], i32, tag="xc")
            cmax32 = xp.tile([P, 2, 32, TL, CH], i32, tag="xc")
            for c in range(CH):
                nc.scalar.activation(
                    cmin32[:, :, :, :, c], E32[:, :, :, :, c], Copy,
                    scale=c_ymB[:, c].unsqueeze(1), bias=float(BIG))
                nc.scalar.activation(
                    cmax32[:, :, :, :, c], E32[:, :, :, :, c], Copy,
                    scale=c_yp1[:, c].unsqueeze(1), bias=0.0)

            # ============== per-frame column tails (DVE trees) =============
            for f in range(TL):
                accT = accTs[f]
                w = 64
                while w >= 2:
                    nc.vector.tensor_tensor(accT[:, :, :, 0:w], accT[:, :, :, 0:w],
                                            accT[:, :, :, w:2 * w], Op.bitwise_or)
                    w //= 2
                # final level lands straight in the shared root buffer
                nc.vector.tensor_tensor(rootc[:, f, :, :], accT[:, :, :, 0],
                                        accT[:, :, :, 1], Op.bitwise_or)

            if dbg:
                nc.sync.dma_start(dbg_cmask[:], rootc[:])

            # row-side min/max via in-place TT trees over the chunk axis
            # (TT consumes 2 inputs/cycle; tensor_reduce only 1)
            w = CH // 2
            while w >= 1:
                nc.vector.tensor_tensor(cmin32[:, :, :, :, 0:w],
                                        cmin32[:, :, :, :, 0:w],
                                        cmin32[:, :, :, :, w:2 * w], Op.min)
                nc.vector.tensor_tensor(cmax32[:, :, :, :, 0:w],
                                        cmax32[:, :, :, :, 0:w],
                                        cmax32[:, :, :, :, w:2 * w], Op.max)
                w //= 2

            # combined signed stat tile: S[p, k, pl, s', f], k: 0=-xmin
            # 1=-ymin 2=xmax+1 3=ymax+1 (max-fold works for all four)
            S = smallp.tile([P, 4, 2, 32, TL], i16)
            ST = smallp.tile([P, 4, 128], i16)
            S2 = S[:].rearrange("p k a b f -> p (k a b f)")   # [128, 512]

            def srow(k, dt=None):
                ap = S[:, k].rearrange("p a b f -> p (a b f)")
                return ap.bitcast(dt) if dt is not None else ap

            def root(t, dt):
                return t[:, :, :, :, 0].rearrange("p a b f -> p (a b f)").bitcast(dt)

            # row stats are ready first: write their S rows and launch their
            # partition-fold transposes while the column side still computes
            nc.vector.tensor_scalar(srow(1), root(cmin32, i32), -1, 0,
                                    Op.mult, Op.add)
            nc.vector.tensor_copy(srow(3), root(cmax32, i32))
            for m in (1, 3):
                nc.sync.dma_start(ST[:, m, :], S2[:, 128 * m:128 * (m + 1)],
                                  transpose=True)

            # ==================== column-side extraction ===================
            # bit-extract on ALL partitions regardless of parity; wrong-parity
            # slots pick 0 from the parity-gated value tables (neutral for
            # both the min and the max trees).
            E16 = xp.tile([P, 2, 32, TL, B], i16, tag="xE")
            for pl in range(2):
                for sp in range(32):
                    bit = (31 - sp) if pl == 0 else sp
                    nc.vector.tensor_scalar(
                        E16[:, pl, sp], rootc[:, :, pl, :].bitcast(i16),
                        bit & 15, 1, Op.logical_shift_right, Op.bitwise_and)

            # value-select via one broadcast TT mult each: cmin16 = E*(v-BIG)
            # in {v-BIG, 0}; 0 neutral for min since v-BIG < 0. cmax16 =
            # E*(v+1); 0 neutral for max.
            cmin16 = xp.tile([P, 2, 32, TL, B], i16, tag="xc")
            cmax16 = xp.tile([P, 2, 32, TL, B], i16, tag="xc")
            bshape = [P, 2, 32, TL, B]
            Tmin = c_xmB4[:].unsqueeze(3).broadcast_to(bshape)
            Tmax = c_xp14[:].unsqueeze(3).broadcast_to(bshape)
            nc.vector.tensor_tensor(cmin16[:], E16[:], Tmin, Op.mult)
            nc.vector.tensor_tensor(cmax16[:], E16[:], Tmax, Op.mult)
            w = B // 2
            while w >= 1:
                nc.vector.tensor_tensor(cmin16[:, :, :, :, 0:w],
                                        cmin16[:, :, :, :, 0:w],
                                        cmin16[:, :, :, :, w:2 * w], Op.min)
                nc.vector.tensor_tensor(cmax16[:, :, :, :, 0:w],
                                        cmax16[:, :, :, :, 0:w],
                                        cmax16[:, :, :, :, w:2 * w], Op.max)
                w //= 2

            # srow0 = -xmin = -(m + BIG) where m = tree-min of E*(v-BIG)
            nc.vector.tensor_scalar(srow(0), root(cmin16, i16), -1, -BIG16,
                                    Op.mult, Op.add)
            nc.vector.tensor_copy(srow(2), root(cmax16, i16))
            for m in (0, 2):
                nc.sync.dma_start(ST[:, m, :], S2[:, 128 * m:128 * (m + 1)],
                                  transpose=True)
            SR = smallp.tile([P, 4], i16)
            nc.vector.tensor_reduce(SR[:], ST[:], axis=X, op=Op.max)

            # finalize: V[p, k] with p = (pl*32+s')*TL + f
            V = smallp.tile([P, 4], i32)
            nc.vector.tensor_copy(V[:], SR[:])
            nc.vector.tensor_scalar(V[:, 0:2], V[:, 0:2], -1, 0, Op.mult, Op.add)
            nc.vector.tensor_scalar(V[:, 2:4], V[:, 2:4], 1, 0, Op.subtract, Op.add)
            BOF = smallp.tile([P, 4], f32)
            fix = smallp.tile([P, 4], f32)
            nc.vector.tensor_copy(BOF[:], V[:])
            # empty segments (in f32, so the sums round exactly to +/-2^31):
            # mins 32767 -> 2147483648.0, maxes -1 -> -2147483648.0
            nc.vector.tensor_scalar(fix[:, 0:2], BOF[:, 0:2], 32767.0, 2147450880.0,
                                    Op.is_equal, Op.mult)
            nc.vector.tensor_scalar(fix[:, 2:4], BOF[:, 2:4], -1.0, -2147483647.0,
                                    Op.is_equal, Op.mult)
            nc.vector.tensor_tensor(BOF[:], BOF[:], fix[:], Op.add)

            # boxes[f, n, k] <- BOF[n*TL + f, k]
            nc.sync.dma_start(boxes_out[:].transpose([1, 0, 2]), BOF[:])

    nc.finalize()
    if split_waits:
        _split_excess_waits(nc, mybir)
    return nc, tables


def _split_excess_waits(nc, mybir):
    """Hoist extra sem waits onto preceding NoOps.

    This walrus build rejects instructions carrying more sync-wait
    conditions than their ISA encoding holds (1 for TPB_CTRL ops and for
    Pool/core_v2 compute ops; 2 elsewhere, conservatively). Semantics are
    identical with the waits split onto dedicated NoOps just before the
    instruction.
    """
    ctrl = {"Drain", "NoOp", "Nop", "EventSemaphore", "AllEngineBarrier"}
    n_split = 0
    for f in nc.m.functions:
        for bb in f.blocks:
            newl = []
            for ins in bb.instructions:
                si = ins.sync_info
                max_waits = 1
                if si and si.on_wait and len(si.on_wait) > max_waits:
                    waits = list(si.on_wait)
                    for j, w in enumerate(waits[max_waits:]):
                        nop = mybir.InstNoOp(
                            name=f"{ins.name}-w{j}", ins=[], outs=[],
                            engine=ins.engine,
                            sync_info=mybir.SyncInfo(on_wait=[w], on_update=[]))
                        newl.append(nop)
                        n_split += 1
                    ins.sync_info = mybir.SyncInfo(on_wait=waits[:max_waits],
                                                   on_update=si.on_update)
                newl.append(ins)
            bb.instructions = newl
    return n_split


def _get_program(TL, H, W, reps=1):
    key = (TL, H, W, reps)
    if key not in _BUILD_CACHE:
        _BUILD_CACHE[key] = _build_program(TL, H, W, reps=reps)
    return _BUILD_CACHE[key]


def kernel(segmentation, num_instances=None, **_ignored):
    from concourse.bass_utils import run_bass_kernel_spmd

    seg = np.asarray(segmentation)
    T, H, W = seg.shape
    assert T % _NCORES == 0
    TL = T // _NCORES
    nc, tables = _get_program(TL, H, W)

    seg = np.ascontiguousarray(seg, dtype=np.int32)
    in_maps = [{"seg": seg[i * TL:(i + 1) * TL], **tables}
               for i in range(_NCORES)]
    res = run_bass_kernel_spmd(nc, in_maps, list(range(_NCORES)))
    out = np.concatenate([res.results[i]["boxes"] for i in range(_NCORES)], axis=0)
    return out.astype(np.float32)


# revision 46
# speedup vs baseline: 232.9598x; 1.0066x over previous
"""Trainium2 Bass kernel for nn_BoxesFromMasks (per-frame segment bounding boxes).

Algorithm (per core, data-parallel over frames):
  For each frame, build per-pixel one-hot bitmasks of the instance id using an
  exponent-bit trick (int ops construct the bit pattern of float 2^k, an ACT
  copy casts float->uint32 which truncates out-of-range ids to 0):
    lo plane: id s in [0,32)  -> bit (31-s)
    hi plane: id s in [32,64) -> bit (s-32)
  Row masks:  OR-reduce each 128-row chunk along the free (column) axis.
  Col masks:  OR-accumulate chunks into a per-column accumulator, then
              DMA-transpose (as uint16) and OR-reduce along rows.
  Extraction: expand mask bits per id with constant tables, select coordinate
              values, min/max reduce, and partition-fold to one partition.

Scheduling: all frames' chunk work is emitted first; the row-side extraction
is emitted before the per-frame column tails so the DVE keeps busy while the
transpose DMAs land; column extraction follows the tails.
"""

import numpy as np

_T, _H, _W, _N = 16, 1024, 2048, 64
_NCORES = 8

_BUILD_CACHE = {}


def _build_program(TL, H, W, split_waits=True, reps=1, dbg=False):
    from contextlib import ExitStack

    import concourse.bass as bass
    import concourse.tile as tile
    import concourse.mybir as mybir
    from concourse.alu_op_type import AluOpType as Op

    f32 = mybir.dt.float32
    i32 = mybir.dt.int32
    u32 = mybir.dt.uint32
    u16 = mybir.dt.uint16
    Copy = mybir.ActivationFunctionType.Copy
    X = mybir.AxisListType.X

    P = 128
    CH = H // P                   # row chunks per frame
    UC = 2 * W                    # u16 columns per plane
    KT = 2 if UC % (128 * 2) == 0 else UC // 128   # transpose DMA splits
    SPLIT = UC // KT              # u16 cols per transpose call
    MPER = SPLIT // 128           # mid-dim blocks per call
    B = UC // 128                 # total transposed blocks
    BIG = 0x7FFF                  # absent sentinel (fits i16, fp32-exact)
    BIG16 = 0x7FFF

    # ---- constant tables ----
    pp = np.arange(P)
    yv = (np.arange(CH)[None, :] * P + pp[:, None]).astype(np.int64)    # [P, CH]
    bb = np.arange(B)
    xv = ((SPLIT // 2) * (bb[None, :] // MPER) + 64 * (bb[None, :] % MPER)
          + (pp[:, None] % 64)).astype(np.int64)                        # [P, B]
    # select-value scalars, fp32 (tensor_scalar AP scalars must be fp32;
    # every value is < 2^15 so fp32 arithmetic on them is exact)
    # column value per un-parity-sorted transposed partition p and block b:
    # u16col = SPLIT*(b//MPER) + 128*(b%MPER) + p, u32 col = u16col >> 1
    xv4 = ((SPLIT * (bb[None, :] // MPER) + 128 * (bb[None, :] % MPER)
            + pp[:, None]) >> 1).astype(np.int64)                       # [P, B]
    # parity gating: slot (pl, sp) reads halfword h=bit>>4; partition p holds
    # halfword p&1 -> wrong-parity entries select 0 (neutral for min and max)
    spv = np.arange(32)
    bit_lo, bit_hi = 31 - spv, spv                                      # [32]
    hreq = np.stack([bit_lo >> 4, bit_hi >> 4], axis=0)                 # [2, 32]
    pargate = (hreq[None, :, :] == (pp[:, None, None] & 1))             # [P, 2, 32]
    xmB4 = np.where(pargate[:, :, :, None], (xv4 - BIG16)[:, None, None, :], 0)
    xp14 = np.where(pargate[:, :, :, None], (xv4 + 1)[:, None, None, :], 0)
    tables = {
        "ymB": (yv - BIG).astype(np.float32),
        "yp1": (yv + 1).astype(np.float32),
        "xmB4": xmB4.astype(np.int16),      # [P, 2, 32, B]
        "xp14": xp14.astype(np.int16),
    }

    nc = bass.Bass()
    seg_in = nc.dram_tensor("seg", [TL, H, W], i32, kind="ExternalInput")
    boxes_out = nc.dram_tensor("boxes", [TL, 64, 4], f32, kind="ExternalOutput")

    i16 = mybir.dt.int16
    d_ymB = nc.dram_tensor("ymB", [P, CH], f32, kind="ExternalInput")
    d_yp1 = nc.dram_tensor("yp1", [P, CH], f32, kind="ExternalInput")
    d_xmB4 = nc.dram_tensor("xmB4", [P, 2, 32, B], i16, kind="ExternalInput")
    d_xp14 = nc.dram_tensor("xp14", [P, 2, 32, B], i16, kind="ExternalInput")

    if dbg:
        dbg_rmask = nc.dram_tensor("dbg_rmask", [P, TL, 2, CH], u32,
                                   kind="ExternalOutput")
        dbg_cmask = nc.dram_tensor("dbg_cmask", [P, TL, 2, B], u16,
                                   kind="ExternalOutput")

    with tile.TileContext(nc) as tc, ExitStack() as ctx:
        constp = ctx.enter_context(tc.tile_pool(name="consts", bufs=1))
        segp = ctx.enter_context(tc.tile_pool(name="segp", bufs=2))
        ep = ctx.enter_context(tc.tile_pool(name="ep", bufs=3))
        accp = ctx.enter_context(tc.tile_pool(name="accp", bufs=2))
        accTp = ctx.enter_context(tc.tile_pool(name="accTp", bufs=2))
        maskp = ctx.enter_context(tc.tile_pool(name="maskp", bufs=1))
        xp = ctx.enter_context(tc.tile_pool(name="xp", bufs=2))
        xc16p = ctx.enter_context(tc.tile_pool(name="xc16", bufs=2))
        trp = ctx.enter_context(tc.tile_pool(name="trp", bufs=2))
        smallp = ctx.enter_context(tc.tile_pool(name="smallp", bufs=1))

        # const tiles declared here, DMAs issued after the first chunk's
        # loads (they are only needed by the extraction phase)
        c_ymB = constp.tile([P, CH], f32)
        c_yp1 = constp.tile([P, CH], f32)
        c_xmB4 = constp.tile([P, 2, 32, B], i16)
        c_xp14 = constp.tile([P, 2, 32, B], i16)

        def load_consts():
            nc.sync.dma_start(c_ymB[:], d_ymB[:])
            nc.sync.dma_start(c_yp1[:], d_yp1[:])
            nc.sync.dma_start(c_xmB4[:], d_xmB4[:])
            nc.sync.dma_start(c_xp14[:], d_xp14[:])

        # body repeated `reps` times (identical output; used for wall-clock
        # device-time measurement: (wall(R) - wall(1)) / (R - 1))
        for _rep in range(reps):
            rmask16 = maskp.tile([P, TL, 2, CH, 16], u32, tag="rmask16")
            # rootc[p, f, pl, b]: column masks at transposed (parity-
            # interleaved) partition order; partition p holds halfword p&1
            rootc = maskp.tile([P, TL, 2, B], u16, tag="rootc")
            # ======== phase A: chunk work, then transposes, per frame ======
            accs, accTs = [], []
            pending = []
            for f in range(TL):
                acc = accp.tile([P, 2, W], u32)
                accs.append(acc)
                prev_u = None
                for c in range(CH):
                    first = (f == 0 and c <= 1)
                    s = segp.tile([P, W], i32)
                    e = ep.tile([P, 2, W], i32)
                    u = e[:].bitcast(u32)  # in-place cast target
                    # lo: bitpattern of 2^(31-s) = (158-s)<<23 ;
                    # hi: 2^(s-32) = (s+95)<<23
                    if first:
                        # ramp: load/generate in two column halves so the DVE
                        # starts ~6us sooner
                        for h in range(2):
                            cl = slice(1024 * h, 1024 * (h + 1))
                            for k in range(4):
                                nc.sync.dma_start(
                                    s[32 * k:32 * (k + 1), cl],
                                    seg_in[f, c * P + 32 * k:c * P + 32 * (k + 1), cl])
                            nc.scalar.activation(e[:, 0, cl], s[:, cl], Copy,
                                                 bias=1325400064.0, scale=-8388608.0)
                            nc.gpsimd.tensor_scalar(e[:, 1, cl], s[:, cl],
                                                    8388608, 796917760,
                                                    Op.mult, Op.add)
                            nc.scalar.activation(u[:, 0, cl], e[:, 0, cl].bitcast(f32),
                                                 Copy)
                            nc.gpsimd.tensor_copy(u[:, 1, cl], e[:, 1, cl].bitcast(f32))
                            if c == 0 and h == 0:
                                load_consts()
                    else:
                        for k in range(4):
                            nc.sync.dma_start(
                                s[32 * k:32 * (k + 1), :],
                                seg_in[f, c * P + 32 * k:c * P + 32 * (k + 1), :])
                        nc.scalar.activation(e[:, 0, :], s[:], Copy,
                                             bias=1325400064.0, scale=-8388608.0)
                        nc.gpsimd.tensor_scalar(e[:, 1, :], s[:], 8388608, 796917760,
                                                Op.mult, Op.add)
                        nc.scalar.activation(u[:, 0, :], e[:, 0, :].bitcast(f32), Copy)
                        nc.gpsimd.tensor_copy(u[:, 1, :], e[:, 1, :].bitcast(f32))

                    # row-mask OR-tree into scratch (u stays intact so the
                    # column accumulate is dependency-independent). DVE ops
                    # are emitted in a rotated order -- the previous chunk's
                    # two deepest tree levels (pending) interleave with this
                    # chunk's shallow levels -- so almost every op's input
                    # dependency is >= 2 ops back and the per-dependency
                    # sem delay is absorbed.
                    tr = trp.tile([P, 2, W // 2], u32)
                    base = tr[:]

                    def lvl(dst, lo, hi):
                        return lambda: nc.vector.tensor_tensor(dst, lo, hi,
                                                               Op.bitwise_or)

                    if first:
                        # per-column-half trees (and, for c==1, half-width
                        # accumulates) so each half's work starts as soon as
                        # its cast lands during the pipeline ramp
                        for h in range(2):
                            hb, ub = 512 * h, 1024 * h
                            if c == 1:
                                cl = slice(1024 * h, 1024 * (h + 1))
                                lvl(acc[:, :, cl], u[:, :, cl],
                                    prev_u[:, :, cl])()
                            elif c > 1:
                                cl = slice(1024 * h, 1024 * (h + 1))
                                lvl(acc[:, :, cl], u[:, :, cl],
                                    acc[:, :, cl])()
                            lvl(base[:, :, hb:hb + 512], u[:, :, ub:ub + 512],
                                u[:, :, ub + 512:ub + 1024])()
                            w = 256
                            while w >= 16:
                                lvl(base[:, :, hb:hb + w], base[:, :, hb:hb + w],
                                    base[:, :, hb + w:hb + 2 * w])()
                                w //= 2
                        lvl(rmask16[:, f, :, c, :], base[:, :, 0:16],
                            base[:, :, 512:528])()
                        prev_u = u
                        continue

                    ops = [
                        lvl(base[:, :, 0:1024], u[:, :, 0:1024], u[:, :, 1024:2048]),
                        lvl(base[:, :, 0:512], base[:, :, 0:512], base[:, :, 512:1024]),
                        lvl(base[:, :, 0:256], base[:, :, 0:256], base[:, :, 256:512]),
                        lvl(base[:, :, 0:128], base[:, :, 0:128], base[:, :, 128:256]),
                        lvl(base[:, :, 0:64], base[:, :, 0:64], base[:, :, 64:128]),
                        lvl(base[:, :, 0:32], base[:, :, 0:32], base[:, :, 32:64]),
                        lvl(rmask16[:, f, :, c, :], base[:, :, 0:16],
                            base[:, :, 16:32]),
                    ]
                    last = (c == CH - 1)
                    if c == 0:
                        prev_u = u
                        accop = None
                    elif last:
                        # final accumulate split by column quarters so the
                        # transposes (gated on the acc) can start per-quarter
                        accop = None
                        accqs = [lvl(acc[:, :, 1024 * q:1024 * (q + 1)],
                                     u[:, :, 1024 * q:1024 * (q + 1)],
                                     acc[:, :, 1024 * q:1024 * (q + 1)])
                                 for q in range(2)]
                    elif c == 1:
                        accop = lvl(acc[:], u, prev_u)
                    else:
                        accop = lvl(acc[:], u, acc[:])

                    ops[0]()                                   # L1
                    if pending:
                        pending[0]()                           # L6 of c-1
                    ops[1]()                                   # L2
                    if pending:
                        pending[1]()                           # L7 of c-1
                    ops[2]()                                   # L3
                    if last:
                        for q in range(2):
                            accqs[q]()
                    elif accop:
                        accop()
                    ops[3]()                                   # L4
                    ops[4]()                                   # L5
                    pending = ops[5:7]

                # flush the final chunk's deep levels before the frame tail
                for op in pending:
                    op()
                pending = []

                # launch this frame's column transposes early (SP stream)
                accT = accTp.tile([P, 2, B, 128], u16)
                accTs.append(accT)
                for pl in range(2):
                    a16 = acc[:, pl, :].bitcast(u16)   # [P, UC]
                    for k in range(KT):
                        nc.sync.dma_start(accT[:, pl, k * MPER:(k + 1) * MPER, :],
                                          a16[:, k * SPLIT:(k + 1) * SPLIT],
                                          transpose=True)

            # ========== row-side extraction (overlaps transposes) ==========
            assert 2 * 32 * TL == 128  # per-stat slot block == transpose col set

            # fold rowmask leftovers [.., 16] -> [.., 1]
            rmf = rmask16[:].rearrange("p a b c w -> p (a b c) w")
            w = 8
            while w >= 1:
                nc.vector.tensor_tensor(rmf[:, :, 0:w], rmf[:, :, 0:w],
                                        rmf[:, :, w:2 * w], Op.bitwise_or)
                w //= 2

            if dbg:
                nc.sync.dma_start(dbg_rmask[:], rmask16[:, :, :, :, 0])

            # E = (mask >> bit) & 1  (one op per id slot; int immediates)
            E32 = xp.tile([P, 2, 32, TL, CH], i32, tag="xE")
            for pl in range(2):
                rm_v = rmask16[:, :, pl, :, 0]            # [P, TL, CH]
                for sp in range(32):
                    bit = (31 - sp) if pl == 0 else sp
                    nc.vector.tensor_scalar(
                        E32[:, pl, sp], rm_v.bitcast(i32), bit, 1,
                        Op.logical_shift_right, Op.bitwise_and)

            # row selects on ACT (overlap the accT trees below):
            # cmin = E*(v-BIG) + BIG in {v, BIG}; cmax = E*(v+1) in {v+1, 0}
            cmin32 = xp.tile([P, 2, 32, TL, CH], i32, tag="xc")
            cmax32 = xp.tile([P, 2, 32, TL, CH], i32, tag="xc")
            for c in range(CH):
                nc.scalar.activation(
                    cmin32[:, :, :, :, c], E32[:, :, :, :, c], Copy,
                    scale=c_ymB[:, c].unsqueeze(1), bias=float(BIG))
                nc.scalar.activation(
                    cmax32[:, :, :, :, c], E32[:, :, :, :, c], Copy,
                    scale=c_yp1[:, c].unsqueeze(1), bias=0.0)

            # ============== per-frame column tails (DVE trees) =============
            for f in range(TL):
                accT = accTs[f]
                w = 64
                while w >= 2:
                    nc.vector.tensor_tensor(accT[:, :, :, 0:w], accT[:, :, :, 0:w],
                                            accT[:, :, :, w:2 * w], Op.bitwise_or)
                    w //= 2
                # final level lands straight in the shared root buffer
                nc.vector.tensor_tensor(rootc[:, f, :, :], accT[:, :, :, 0],
                                        accT[:, :, :, 1], Op.bitwise_or)

            if dbg:
                nc.sync.dma_start(dbg_cmask[:], rootc[:])

            # row-side min/max via in-place TT trees over the chunk axis
            # (TT consumes 2 inputs/cycle; tensor_reduce only 1)
            w = CH // 2
            while w >= 1:
                nc.vector.tensor_tensor(cmin32[:, :, :, :, 0:w],
                                        cmin32[:, :, :, :, 0:w],
                                        cmin32[:, :, :, :, w:2 * w], Op.min)
                nc.vector.tensor_tensor(cmax32[:, :, :, :, 0:w],
                                        cmax32[:, :, :, :, 0:w],
                                        cmax32[:, :, :, :, w:2 * w], Op.max)
                w //= 2

            # combined signed stat tile: S[p, k, pl, s', f], k: 0=-xmin
            # 1=-ymin 2=xmax+1 3=ymax+1 (max-fold works for all four)
            S = smallp.tile([P, 4, 2, 32, TL], i16)
            ST = smallp.tile([P, 4, 128], i16)
            S2 = S[:].rearrange("p k a b f -> p (k a b f)")   # [128, 512]

            def srow(k, dt=None):
                ap = S[:, k].rearrange("p a b f -> p (a b f)")
                return ap.bitcast(dt) if dt is not None else ap

            def root(t, dt):
                return t[:, :, :, :, 0].rearrange("p a b f -> p (a b f)").bitcast(dt)

            # row stats are ready first: write their S rows and launch their
            # partition-fold transposes while the column side still computes
            nc.vector.tensor_scalar(srow(1), root(cmin32, i32), -1, 0,
                                    Op.mult, Op.add)
            nc.vector.tensor_copy(srow(3), root(cmax32, i32))
            for m in (1, 3):
                nc.sync.dma_start(ST[:, m, :], S2[:, 128 * m:128 * (m + 1)],
                                  transpose=True)

            # ==================== column-side extraction ===================
            # bit-extract on ALL partitions regardless of parity; wrong-parity
            # slots pick 0 from the parity-gated value tables (neutral for
            # both the min and the max trees).
            E16 = xp.tile([P, 2, 32, TL, B], i16, tag="xE")
            for pl in range(2):
                for sp in range(32):
                    bit = (31 - sp) if pl == 0 else sp
                    nc.vector.tensor_scalar(
                        E16[:, pl, sp], rootc[:, :, pl, :].bitcast(i16),
                        bit & 15, 1, Op.logical_shift_right, Op.bitwise_and)

            # value-select via one broadcast TT mult each: cmin16 = E*(v-BIG)
            # in {v-BIG, 0}; 0 neutral for min since v-BIG < 0. cmax16 =
            # E*(v+1); 0 neutral for max.
            cmin16 = xp.tile([P, 2, 32, TL, B], i16, tag="xc")
            cmax16 = xp.tile([P, 2, 32, TL, B], i16, tag="xc")
            bshape = [P, 2, 32, TL, B]
            Tmin = c_xmB4[:].unsqueeze(3).broadcast_to(bshape)
            Tmax = c_xp14[:].unsqueeze(3).broadcast_to(bshape)
            nc.vector.tensor_tensor(cmin16[:], E16[:], Tmin, Op.mult)
            nc.vector.tensor_tensor(cmax16[:], E16[:], Tmax, Op.mult)
            w = B // 2
            while w >= 1:
                nc.vector.tensor_tensor(cmin16[:, :, :, :, 0:w],
                                        cmin16[:, :, :, :, 0:w],
                                        cmin16[:, :, :, :, w:2 * w], Op.min)
                nc.vector.tensor_tensor(cmax16[:, :, :, :, 0:w],
                                        cmax16[:, :, :, :, 0:w],
                                        cmax16[:, :, :, :, w:2 * w], Op.max)
                w //= 2

            # srow0 = -xmin = -(m + BIG) where m = tree-min of E*(v-BIG)
            nc.vector.tensor_scalar(srow(0), root(cmin16, i16), -1, -BIG16,
                                    Op.mult, Op.add)
            nc.vector.tensor_copy(srow(2), root(cmax16, i16))
            for m in (0, 2):
                nc.sync.dma_start(ST[:, m, :], S2[:, 128 * m:128 * (m + 1)],
                                  transpose=True)
            SR = smallp.tile([P, 4], i16)
            nc.vector.tensor_reduce(SR[:], ST[:], axis=X, op=Op.max)

            # finalize: V[p, k] with p = (pl*32+s')*TL + f
            V = smallp.tile([P, 4], i32)
            nc.vector.tensor_copy(V[:], SR[:])
            nc.vector.tensor_scalar(V[:, 0:2], V[:, 0:2], -1, 0, Op.mult, Op.add)
            nc.vector.tensor_scalar(V[:, 2:4], V[:, 2:4], 1, 0, Op.subtract, Op.add)
            BOF = smallp.tile([P, 4], f32)
            fix = smallp.tile([P, 4], f32)
            nc.vector.tensor_copy(BOF[:], V[:])
            # empty segments (in f32, so the sums round exactly to +/-2^31):
            # mins 32767 -> 2147483648.0, maxes -1 -> -2147483648.0
            nc.vector.tensor_scalar(fix[:, 0:2], BOF[:, 0:2], 32767.0, 2147450880.0,
                                    Op.is_equal, Op.mult)
            nc.vector.tensor_scalar(fix[:, 2:4], BOF[:, 2:4], -1.0, -2147483647.0,
                                    Op.is_equal, Op.mult)
            nc.vector.tensor_tensor(BOF[:], BOF[:], fix[:], Op.add)

            # boxes[f, n, k] <- BOF[n*TL + f, k]
            nc.sync.dma_start(boxes_out[:].transpose([1, 0, 2]), BOF[:])

    nc.finalize()
    if split_waits:
        _split_excess_waits(nc, mybir)
    return nc, tables


def _split_excess_waits(nc, mybir):
    """Hoist extra sem waits onto preceding NoOps.

    This walrus build rejects instructions carrying more sync-wait
    conditions than their ISA encoding holds (1 for TPB_CTRL ops and for
    Pool/core_v2 compute ops; 2 elsewhere, conservatively). Semantics are
    identical with the waits split onto dedicated NoOps just before the
    instruction.
    """
    ctrl = {"Drain", "NoOp", "Nop", "EventSemaphore", "AllEngineBarrier"}
    n_split = 0
    for f in nc.m.functions:
        for bb in f.blocks:
            newl = []
            for ins in bb.instructions:
                si = ins.sync_info
                max_waits = 1
                if si and si.on_wait and len(si.on_wait) > max_waits:
                    waits = list(si.on_wait)
                    for j, w in enumerate(waits[max_waits:]):
                        nop = mybir.InstNoOp(
                            name=f"{ins.name}-w{j}", ins=[], outs=[],
                            engine=ins.engine,
                            sync_info=mybir.SyncInfo(on_wait=[w], on_update=[]))
                        newl.append(nop)
                        n_split += 1
                    ins.sync_info = mybir.SyncInfo(on_wait=waits[:max_waits],
                                                   on_update=si.on_update)
                newl.append(ins)
            bb.instructions = newl
    return n_split


def _get_program(TL, H, W, reps=1):
    key = (TL, H, W, reps)
    if key not in _BUILD_CACHE:
        _BUILD_CACHE[key] = _build_program(TL, H, W, reps=reps)
    return _BUILD_CACHE[key]


def kernel(segmentation, num_instances=None, **_ignored):
    from concourse.bass_utils import run_bass_kernel_spmd

    seg = np.asarray(segmentation)
    T, H, W = seg.shape
    assert T % _NCORES == 0
    TL = T // _NCORES
    nc, tables = _get_program(TL, H, W)

    seg = np.ascontiguousarray(seg, dtype=np.int32)
    in_maps = [{"seg": seg[i * TL:(i + 1) * TL], **tables}
               for i in range(_NCORES)]
    res = run_bass_kernel_spmd(nc, in_maps, list(range(_NCORES)))
    out = np.concatenate([res.results[i]["boxes"] for i in range(_NCORES)], axis=0)
    return out.astype(np.float32)
